# revision 1
# baseline (speedup 1.0000x reference)
"""Trainium2 Bass kernel for nn_Encoder (GNN message passing, PDP-VRP encoder).

Sharding: 2 graphs per core x 8 cores. The per-layer cross-graph row scramble
(faithful torch.cat(dim=0).view) is handled with a ReduceScatter exchange in
global-flat row order. BatchNorm batch stats via moment-matrix AllReduce.
All conv compute in feature-major (transposed) layout:
  psum[h, (j,i)] = we~.T @ E_aug (+BN fold, mask fold as extra K rows)
                 + wi.T @ x bcast-over-i + wj.T @ x bcast-over-j
  P = exp(prelu(psum)) fp16; D = sum_i P; N = sum_i P*x_i; out = N/D.
"""
import numpy as np

B, D, NN = 16, 2, 100
N2, NA = 50, 102
H, HE, L = 128, 64, 3
SLOPE, EPS = 0.2, 1e-5
NCORE = 8
BL = 2                     # graphs per core
COLS = BL * NA * NA        # 20808 edge cols per chain per core
MB = -200.0                # additive mask constant
FLAT = B * (NA + 2 * N2)   # 3232 global flat rows
WIN = FLAT // NCORE        # 404 rows per core window
ECH = 1536                 # embed streaming chunk

_CACHE = {}


def _chunks_full():
    return [(j, 5) for j in range(0, 100, 5)] + [(100, 2)]


def _chunks_sub():
    return [(j, 10) for j in range(0, 50, 10)]


def _groups(chunks, n=3):
    return [chunks[i:i + n] for i in range(0, len(chunks), n)]


def build(gpsimd_offload=True, emulate_collectives=False):
    import concourse.bass as bass
    import concourse.bacc as bacc
    import concourse.tile as tile
    import concourse.mybir as mybir
    from concourse import masks

    dt = mybir.dt
    F32, F16 = dt.float32, dt.float16
    AF = mybir.ActivationFunctionType
    OP = mybir.AluOpType
    AX = mybir.AxisListType

    nc = bacc.Bacc("TRN2", target_bir_lowering=False, debug=False,
                   num_devices=NCORE)

    def din(name, shape, d=F32):
        return nc.dram_tensor(name, shape, d, kind="ExternalInput").ap()

    dsT = din("dsT", [5, BL * NA])
    pkinT = din("pkinT", [10, BL * N2])
    dep_nat = din("dep_nat", [BL * D, 6])
    pk_nat = din("pk_nat", [BL * N2, 11])
    dl_nat = din("dl_nat", [BL * N2, 6])
    eT = {c: din(f"eT_{c}", [2, COLS], F16) for c in "dr"}
    e_nat = {c: din(f"e_nat_{c}", [128, 163 * 3]) for c in "dr"}
    m_in = {c: din(f"m_{c}", [BL, NA * NA], F16) for c in "dr"}
    W0 = din("W0", [5, H]); W1 = din("W1", [10, H]); W2 = din("W2", [5, H])
    W3 = din("W3", [2, HE]); W4 = din("W4", [2, HE])
    gb = {}
    for i, hh in [(0, H), (1, H), (2, H), (3, HE), (4, HE)]:
        gb[f"b{i}_g"] = din(f"b{i}_g", [hh])
        gb[f"b{i}_b"] = din(f"b{i}_b", [hh])
    Wvl = {k: din(f"Wvl{k}", [L, H, H]) for k in "apd"}
    Wgx = {k: din(f"Wg{k}", [L, 2 * H + HE, H]) for k in "apd"}
    ff_w1 = din("ff_w1", [H, H]); ff_w2 = din("ff_w2", [H, H])
    ff_b1 = din("ff_b1", [H]); ff_b2 = din("ff_b2", [H])
    bn_g = din("bn_g", [H]); bn_b = din("bn_b", [H])

    o_out = {c: nc.dram_tensor(f"o_{c}", [BL, NA, H], F32,
                               kind="ExternalOutput").ap() for c in "dr"}

    E_st = {c: nc.dram_tensor(f"E_{c}", [67, BL, NA, NA], F16).ap()
            for c in "dr"}
    rs_in = {(c, l): nc.dram_tensor(f"rsi_{c}{l}", [FLAT, H], F32).ap()
             for c in "dr" for l in range(L)}
    rs_out = {(c, l): nc.dram_tensor(f"rso_{c}{l}", [WIN, H], F32).ap()
              for c in "dr" for l in range(L)}
    ar1_i = nc.dram_tensor("ar1_i", [128, 16], F32).ap()
    ar1_o = nc.dram_tensor("ar1_o", [128, 16], F32).ap()
    ar2_i = nc.dram_tensor("ar2_i", [128, 8], F32).ap()
    ar2_o = nc.dram_tensor("ar2_o", [128, 8], F32).ap()
    GRP = [list(range(NCORE))]

    import contextlib
    with tile.TileContext(nc) as tc, contextlib.ExitStack() as ctx:
        cpool = ctx.enter_context(tc.tile_pool(name="const", bufs=1))
        wk = ctx.enter_context(tc.tile_pool(name="work", bufs=3))
        xpool = ctx.enter_context(tc.tile_pool(name="xt", bufs=6))
        epool = ctx.enter_context(tc.tile_pool(name="eg", bufs=3))
        fpool = ctx.enter_context(tc.tile_pool(name="f16", bufs=3))
        ps_b = ctx.enter_context(tc.tile_pool(name="psb", bufs=2, space="PSUM"))
        ps_s = ctx.enter_context(tc.tile_pool(name="pss", bufs=2, space="PSUM"))

        def ctile(shape, d, tag):
            return cpool.tile(shape, d, tag=tag, name=tag)

        ident = ctile([128, 128], F32, "ident")
        masks.make_identity(nc, ident[:])

        def col(ap_1d, hh, tag):
            t = ctile([hh, 1], F32, tag)
            nc.sync.dma_start(t[:], ap_1d.unsqueeze(1))
            return t

        W0s = ctile([5, H], F32, "W0s"); nc.sync.dma_start(W0s[:], W0[:])
        W1s = ctile([10, H], F32, "W1s"); nc.sync.dma_start(W1s[:], W1[:])
        W2s = ctile([5, H], F32, "W2s"); nc.sync.dma_start(W2s[:], W2[:])
        W3s = ctile([2, HE], F32, "W3s"); nc.sync.dma_start(W3s[:], W3[:])
        W4s = ctile([2, HE], F32, "W4s"); nc.sync.dma_start(W4s[:], W4[:])
        Wes = {"d": W3s, "r": W4s}
        W3h = ctile([2, HE], F16, "W3h")
        nc.vector.tensor_copy(W3h[:], W3s[:])
        W4h = ctile([2, HE], F16, "W4h")
        nc.vector.tensor_copy(W4h[:], W4s[:])
        Wesh = {"d": W3h, "r": W4h}
        wv, wi_s, wj_s, we_s = {}, {}, {}, {}
        for k in "apd":
            for l in range(L):
                t = ctile([H, H], F32, f"wv{k}{l}")
                nc.sync.dma_start(t[:], Wvl[k][l])
                wv[(k, l)] = t
                ti = ctile([H, H], F32, f"wi{k}{l}")
                nc.sync.dma_start(ti[:], Wgx[k][l, 0:H, :])
                tir = ctile([H, H], dt.float32r, f"wir{k}{l}")
                nc.vector.tensor_copy(tir[:], ti[:])
                wi_s[(k, l)] = tir
                tj = ctile([H, H], F32, f"wj{k}{l}")
                nc.sync.dma_start(tj[:], Wgx[k][l, H:2 * H, :])
                tjr = ctile([H, H], dt.float32r, f"wjr{k}{l}")
                nc.vector.tensor_copy(tjr[:], tj[:])
                wj_s[(k, l)] = tjr
                te = ctile([HE, H], F32, f"we{k}{l}")
                nc.sync.dma_start(te[:], Wgx[k][l, 2 * H:2 * H + HE, :])
                we_s[(k, l)] = te
        ffw1 = ctile([H, H], F32, "ffw1"); nc.sync.dma_start(ffw1[:], ff_w1[:])
        ffw2 = ctile([H, H], F32, "ffw2"); nc.sync.dma_start(ffw2[:], ff_w2[:])
        ffb1c = col(ff_b1[:], H, "ffb1")
        ffb2c = col(ff_b2[:], H, "ffb2")
        bngc = col(bn_g[:], H, "bng")
        bnbc = col(bn_b[:], H, "bnb")
        gbc = {k: col(v[:], v.shape[0], f"c{k}") for k, v in gb.items()}

        zero128 = ctile([128, 128], F32, "zero")
        nc.vector.memset(zero128[:], 0.0)
        ones_row = ctile([1, ECH], F16, "ones")
        nc.vector.memset(ones_row[:], 1.0)
        row_p200 = ctile([1, H], F16, "rowp200")
        nc.vector.memset(row_p200[:], -MB)
        row_m200 = ctile([1, H], F16, "rowm200")
        nc.vector.memset(row_m200[:], MB)

        zrep = zero128[:].unsqueeze(1).broadcast_to([128, 25, 128])
        for key, t in rs_in.items():
            nc.scalar.dma_start(t[0:3200, :].rearrange("(a b) h -> b a h", b=128),
                                zrep)
            nc.scalar.dma_start(t[3200:FLAT, :], zero128[0:32, :])

        with tc.tile_critical():
            pid = nc.gpsimd.partition_id()

        # ---------------- embeddings & stats ----------------
        stats = ctile([128, 16], F32, "stats")
        nc.vector.memset(stats[:], 0.0)

        def evac(ps_ap, hh, wid, tag, d=F32):
            t = wk.tile([hh, wid], d, tag=tag)
            nc.scalar.copy(t[:], ps_ap)
            return t

        def moments(nat_ap, rows, fdim):
            nt = wk.tile([rows, fdim + 1], F32, tag=f"nat{fdim}")
            nc.sync.dma_start(nt[:], nat_ap[:])
            ps = ps_s.tile([fdim, fdim + 1], F32, tag="pss")
            nc.tensor.matmul(ps[:], nt[:, 0:fdim], nt[:], start=True, stop=True)
            return evac(ps[:], fdim, fdim + 1, f"mom{fdim}")

        def w_transpose(w, kdim, hh):
            ps = ps_s.tile([hh, kdim], F32, tag="pss")
            nc.tensor.transpose(ps[:], w[:], ident[:kdim, :kdim])
            return evac(ps[:], hh, kdim, "wT")

        def embed_stats(mom, Ws, kdim, hh, scol):
            ps = ps_s.tile([hh, 1], F32, tag="pss")
            nc.tensor.matmul(ps[:], Ws[:], mom[:, kdim:kdim + 1],
                             start=True, stop=True)
            nc.vector.tensor_copy(stats[0:hh, scol:scol + 1], ps[:])
            ps2 = ps_s.tile([hh, kdim], F32, tag="pss")
            nc.tensor.matmul(ps2[:], Ws[:], mom[:, 0:kdim],
                             start=True, stop=True)
            G2 = evac(ps2[:], hh, kdim, "G2")
            WT = w_transpose(Ws, kdim, hh)
            prod = wk.tile([hh, kdim], F32, tag="prod")
            nc.vector.tensor_tensor(prod[:], G2[:], WT[:], op=OP.mult)
            nc.vector.tensor_reduce(stats[0:hh, scol + 1:scol + 2], prod[:],
                                    axis=AX.X, op=OP.add)

        embed_stats(moments(dep_nat, BL * D, 5), W0s, 5, H, 0)
        embed_stats(moments(pk_nat, BL * N2, 10), W1s, 10, H, 2)
        embed_stats(moments(dl_nat, BL * N2, 5), W2s, 5, H, 4)

        for ci, c in enumerate("dr"):
            na = ctile([128, 163 * 3], F32, f"enat{c}")
            nc.sync.dma_start(na[:], e_nat[c][:])
            nav = na[:].rearrange("p (n c) -> p n c", n=163)
            ps = ps_s.tile([2, 3], F32, tag="pss")
            for n in range(163):
                nc.tensor.matmul(ps[:], nav[:, n, 0:2], nav[:, n, :],
                                 start=(n == 0), stop=(n == 162))
            mom = evac(ps[:], 2, 3, "mome")
            embed_stats(mom, Wes[c], 2, HE, 6 + 2 * ci)

        # edge embedding -> E_st rows 0..63 raw z fp16; row 64 mask; row 65 ones
        for c in "dr":
            dst = E_st[c].rearrange("r b j i -> r (b j i)")
            for c0 in range(0, COLS, ECH):
                CH = min(ECH, COLS - c0)
                et = wk.tile([2, ECH], F16, tag="etch", bufs=2)
                nc.sync.dma_start(et[:, 0:CH], eT[c][:, c0:c0 + CH])
                psg = ps_b.tile([128, 1536], F32, tag="psg")
                for k in range((CH + 511) // 512):
                    w = min(512, CH - k * 512)
                    nc.tensor.matmul(psg[0:64, k * 512:k * 512 + w],
                                     Wesh[c][:],
                                     et[:, k * 512:k * 512 + w],
                                     start=True, stop=True)
                ez = fpool.tile([64, ECH], F16, tag="ez", bufs=2)
                nc.scalar.copy(ez[:, 0:CH], psg[0:64, 0:CH])
                nc.sync.dma_start(dst[0:64, c0:c0 + CH], ez[:, 0:CH])
                nc.sync.dma_start(dst[65:66, c0:c0 + CH], ones_row[:, 0:CH])
                nc.sync.dma_start(dst[66:67, c0:c0 + CH], ones_row[:, 0:CH])
            nc.sync.dma_start(dst[64:65, :],
                              m_in[c][:].rearrange("g n -> (g n)").unsqueeze(0))

        nc.sync.dma_start(ar1_i[:], stats[:])
        if emulate_collectives:
            nc.sync.dma_start(ar1_o[:], ar1_i[:])
        else:
            nc.gpsimd.collective_compute("AllReduce", OP.add, replica_groups=GRP,
                                         ins=[ar1_i], outs=[ar1_o])
        sts = ctile([128, 16], F32, "sts")
        nc.sync.dma_start(sts[:], ar1_o[:])

        def bn_vecs(src, scol, n, gc, bc, hh, tag):
            inv = 1.0 / n
            m = wk.tile([hh, 1], F32, tag=f"m{tag}")
            nc.vector.tensor_scalar_mul(m[:], src[0:hh, scol:scol + 1], inv)
            v = wk.tile([hh, 1], F32, tag=f"v{tag}")
            nc.vector.tensor_scalar_mul(v[:], src[0:hh, scol + 1:scol + 2], inv)
            msq = wk.tile([hh, 1], F32, tag=f"q{tag}")
            nc.vector.tensor_tensor(msq[:], m[:], m[:], op=OP.mult)
            nc.vector.tensor_tensor(v[:], v[:], msq[:], op=OP.subtract)
            nc.vector.tensor_scalar_add(v[:], v[:], EPS)
            sd = wk.tile([hh, 1], F32, tag=f"s{tag}")
            nc.scalar.activation(sd[:], v[:], AF.Sqrt)
            rsd = wk.tile([hh, 1], F32, tag=f"r{tag}")
            nc.vector.reciprocal(rsd[:], sd[:])
            sc = ctile([hh, 1], F32, f"sc{tag}")
            nc.vector.tensor_tensor(sc[:], rsd[:], gc[:], op=OP.mult)
            sh = ctile([hh, 1], F32, f"sh{tag}")
            nc.vector.tensor_tensor(sh[:], m[:], sc[:], op=OP.mult)
            nc.vector.tensor_tensor(sh[:], bc[:], sh[:], op=OP.subtract)
            return sc, sh

        sc0, sh0 = bn_vecs(sts, 0, B * D, gbc["b0_g"], gbc["b0_b"], H, "b0")
        sc1, sh1 = bn_vecs(sts, 2, B * N2, gbc["b1_g"], gbc["b1_b"], H, "b1")
        sc2, sh2 = bn_vecs(sts, 4, B * N2, gbc["b2_g"], gbc["b2_b"], H, "b2")
        sce, she = {}, {}
        sce["d"], she["d"] = bn_vecs(sts, 6, B * NA * NA, gbc["b3_g"],
                                     gbc["b3_b"], HE, "b3")
        sce["r"], she["r"] = bn_vecs(sts, 8, B * NA * NA, gbc["b4_g"],
                                     gbc["b4_b"], HE, "b4")

        F16_ = F16
        lhsT_aug = {}
        for c in "dr":
            for k in "apd":
                for l in range(L):
                    t = ctile([67, H], F16_, f"la{c}{k}{l}")
                    nc.vector.tensor_scalar(t[0:64, :], we_s[(k, l)][:],
                                            sce[c][:], None, op0=OP.mult)
                    nc.sync.dma_start(t[64:65, :], row_p200[:])
                    nc.sync.dma_start(t[66:67, :], row_m200[:])
                    ps = ps_s.tile([H, 1], F32, tag="pss")
                    nc.tensor.matmul(ps[:], we_s[(k, l)][:], she[c][:],
                                     start=True, stop=True)
                    cc = evac(ps[:], H, 1, "cc")
                    ps2 = ps_s.tile([1, H], F32, tag="pss")
                    nc.tensor.transpose(ps2[:], cc[:], ident[:])
                    crow = wk.tile([1, H], F16, tag="crow")
                    nc.scalar.copy(crow[:], ps2[:])
                    nc.sync.dma_start(t[65:66, :], crow[:])
                    lhsT_aug[(c, k, l)] = t

        # node embeddings -> xT0
        xT0 = xpool.tile([H, BL * NA], F32, tag="xT")
        dsTs = wk.tile([5, BL * NA], F32, tag="dsTs")
        nc.sync.dma_start(dsTs[:], dsT[:])
        dsv = dsTs[:].rearrange("p (g n) -> p g n", g=BL)
        x0v = xT0[:].rearrange("p (g n) -> p g n", g=BL)
        ps = ps_s.tile([H, BL * D], F32, tag="pss")
        nc.tensor.matmul(ps[:], W0s[:], dsv[:, :, 0:D], start=True, stop=True)
        nc.vector.tensor_scalar(
            x0v[:, :, 0:D], ps[:].rearrange("p (g n) -> p g n", g=BL),
            sc0[:], sh0[:], op0=OP.mult, op1=OP.add)
        pkt = wk.tile([10, BL * N2], F32, tag="pkt")
        nc.sync.dma_start(pkt[:], pkinT[:])
        ps = ps_s.tile([H, BL * N2], F32, tag="pss")
        nc.tensor.matmul(ps[:], W1s[:], pkt[:], start=True, stop=True)
        nc.vector.tensor_scalar(
            x0v[:, :, D:D + N2], ps[:].rearrange("p (g n) -> p g n", g=BL),
            sc1[:], sh1[:], op0=OP.mult, op1=OP.add)
        ps = ps_s.tile([H, BL * N2], F32, tag="pss")
        nc.tensor.matmul(ps[:], W2s[:], dsv[:, :, D + N2:NA],
                         start=True, stop=True)
        nc.vector.tensor_scalar(
            x0v[:, :, D + N2:NA], ps[:].rearrange("p (g n) -> p g n", g=BL),
            sc2[:], sh2[:], op0=OP.mult, op1=OP.add)

        # ---------------- conv layers ----------------
        def mm_evac(w, rhs_ap, wid, tag):
            ps = ps_s.tile([H, wid], F32, tag="pss")
            nc.tensor.matmul(ps[:], w[:], rhs_ap, start=True, stop=True)
            t = xpool.tile([H, wid], dt.float32r, tag=tag)
            nc.scalar.copy(t[:], ps[:])
            th = xpool.tile([H, wid], F16, tag=tag + "h")
            nc.vector.tensor_copy(th[:], t[:])
            return t, th

        def conv_units(c, l, xTin):
            xv = xTin[:].rearrange("p (g n) -> p g n", g=BL)
            xall, xallh = mm_evac(wv[("a", l)], xTin[:], BL * NA, "xa")
            pick, pickh = mm_evac(wv[("p", l)], xv[:, :, D:D + N2],
                                  BL * N2, "xp")
            deli, delih = mm_evac(wv[("d", l)], xv[:, :, D + N2:NA],
                                  BL * N2, "xd")
            cfg = [("a", xall, xallh, NA, _chunks_full()),
                   ("p", pick, pickh, N2, _chunks_sub()),
                   ("d", deli, delih, N2, _chunks_sub())]
            units = []
            for k_, xk_, xkh_, S_, chunks_ in cfg:
                for g_ in range(BL):
                    units.append((k_, xk_, xkh_, S_, chunks_, g_))

            def emit_unit(u):
                k, xk, xkh, S, chunks, g = u
                ilen = S
                la = lhsT_aug[(c, k, l)]
                wi, wjt = wi_s[(k, l)], wj_s[(k, l)]
                if True:
                    Dt = wk.tile([H, S], F16, tag="Dt")
                    Nt = wk.tile([H, S], F16, tag="Nt")
                    Eg = epool.tile([67, S * S], F16, tag=f"Eg{k}", bufs=2)
                    if k == "a":
                        esrc = E_st[c][:, g, :, :]
                    elif k == "p":
                        esrc = E_st[c][:, g, D:D + N2, D:D + N2]
                    else:
                        esrc = E_st[c][:, g, D + N2:NA, D + N2:NA]
                    nc.sync.dma_start(Eg[:, 0:S * S], esrc)
                    for grp in _groups(chunks):
                        njtot = sum(nj for _, nj in grp)
                        j0g = grp[0][0]
                        psg = ps_b.tile([128, 1536], F32, tag="psg")
                        eoff = j0g * ilen
                        for ki, (j0, nj) in enumerate(grp):
                            nc.tensor.matmul(
                                psg[:, ki * 512:ki * 512 + nj * ilen],
                                la[:], Eg[:, eoff:eoff + nj * ilen],
                                start=True, stop=False)
                            eoff += nj * ilen
                        for ki, (j0, nj) in enumerate(grp):
                            a_rhs = xk[:, g * S + j0:g * S + j0 + nj]\
                                .unsqueeze(2).broadcast_to([H, nj, ilen])
                            nc.tensor.matmul(
                                psg[:, ki * 512:ki * 512 + nj * ilen],
                                wi[:], a_rhs, start=False, stop=False)
                        b_base = xk[:, g * S:g * S + ilen]
                        for ki, (j0, nj) in enumerate(grp):
                            b_rhs = b_base.unsqueeze(1)\
                                .broadcast_to([H, nj, ilen])
                            nc.tensor.matmul(
                                psg[:, ki * 512:ki * 512 + nj * ilen],
                                wjt[:], b_rhs, start=False, stop=True)
                        fd = (len(grp) - 1) * 512 + grp[-1][1] * ilen
                        tg = fpool.tile([128, 1536], F16, tag="tg", bufs=2)
                        nc.scalar.activation(tg[:, 0:fd], psg[:, 0:fd],
                                             AF.Prelu, alpha=SLOPE)
                        Pg = fpool.tile([128, 1536], F16, tag="Pg")
                        nc.scalar.activation(Pg[:, 0:fd], tg[:, 0:fd], AF.Exp)
                        wg = fpool.tile([128, 1536], F16, tag="wg", bufs=2)
                        xkb = xkh[:, g * S:g * S + ilen]
                        runs = []
                        for ki, (j0, nj) in enumerate(grp):
                            if runs and runs[-1][2] == nj:
                                runs[-1][1] += 1
                            else:
                                runs.append([ki, 1, nj, j0])
                        hl = ilen // 2  # 51 or 25
                        pf = fpool.tile([128, 768], F16, tag="pf", bufs=2)
                        wf = fpool.tile([128, 768], F16, tag="wf", bufs=2)
                        for k0, nk, nj, j0r in runs:
                            base = Pg[:, k0 * 512:(k0 + nk) * 512]
                            wbase = wg[:, k0 * 512:(k0 + nk) * 512]
                            pv4 = base.rearrange("p (k r) -> p k r", k=nk)\
                                [:, :, 0:nj * ilen].rearrange(
                                "p k (j i) -> p k j i", i=ilen)
                            wv4 = wbase.rearrange("p (k r) -> p k r", k=nk)\
                                [:, :, 0:nj * ilen].rearrange(
                                "p k (j i) -> p k j i", i=ilen)
                            xb4 = xkb.unsqueeze(1).unsqueeze(1)\
                                .broadcast_to([H, nk, nj, ilen])
                            nc.vector.tensor_tensor(wv4, pv4, xb4, op=OP.mult)
                            pf4 = pf[:, k0 * 256:k0 * 256 + nk * nj * hl]\
                                .rearrange("p (k j i) -> p k j i", k=nk, j=nj)
                            wf4 = wf[:, k0 * 256:k0 * 256 + nk * nj * hl]\
                                .rearrange("p (k j i) -> p k j i", k=nk, j=nj)
                            nc.vector.tensor_tensor(
                                pf4, pv4[:, :, :, 0:hl],
                                pv4[:, :, :, hl:2 * hl], op=OP.add)
                            eng = nc.gpsimd if gpsimd_offload else nc.vector
                            eng.tensor_tensor(
                                wf4, wv4[:, :, :, 0:hl],
                                wv4[:, :, :, hl:2 * hl], op=OP.add)
                            dv = Dt[:, j0r:j0r + nk * nj].rearrange(
                                "p (k j) -> p k j", k=nk)
                            nv = Nt[:, j0r:j0r + nk * nj].rearrange(
                                "p (k j) -> p k j", k=nk)
                            with nc.allow_low_precision("fp16 softmax sums"):
                                nc.vector.tensor_reduce(dv, pf4, axis=AX.X,
                                                        op=OP.add)
                                nc.vector.tensor_reduce(nv, wf4, axis=AX.X,
                                                        op=OP.add)
                    Di = wk.tile([H, S], F32, tag="Di")
                    nc.vector.reciprocal(Di[:], Dt[:])
                    og = wk.tile([H, S], F32, tag="og")
                    nc.vector.tensor_tensor(og[:], Nt[:], Di[:], op=OP.mult)
                    pst = ps_s.tile([S, H], F32, tag="pss")
                    nc.tensor.transpose(pst[:], og[:], ident[:])
                    nat2 = wk.tile([S, H], F32, tag="nat2")
                    nc.scalar.copy(nat2[:], pst[:])
                    if k == "a":
                        off = pid * (BL * NA) + g * NA
                    elif k == "p":
                        off = pid * (BL * N2) + g * N2 + B * NA
                    else:
                        off = pid * (BL * N2) + g * N2 + B * (NA + N2)
                    nc.gpsimd.dma_start(rs_in[(c, l)][bass.ds(off, S), :],
                                        nat2[:])
            def emit_rs():
                if emulate_collectives:
                    nc.sync.dma_start(rs_out[(c, l)][:], rs_in[(c, l)][0:WIN, :])
                else:
                    nc.gpsimd.collective_compute(
                        "ReduceScatter", OP.add, replica_groups=GRP,
                        ins=[rs_in[(c, l)]], outs=[rs_out[(c, l)]])
            return [lambda u=u: emit_unit(u) for u in units] + [emit_rs]

        def assemble(c, l):
            xn = xpool.tile([H, BL * NA], F32, tag="xT")
            for g in range(BL):
                n1 = wk.tile([128, H], F32, tag="asm")
                nc.sync.dma_start(n1[:],
                                  rs_out[(c, l)][g * 202:g * 202 + 128, :])
                n2 = wk.tile([128, H], F32, tag="asm")
                nc.sync.dma_start(
                    n2[0:74, :], rs_out[(c, l)][g * 202 + 128:g * 202 + 202, :])
                p1 = ps_s.tile([H, 128], F32, tag="pss")
                nc.tensor.transpose(p1[:], n1[:], ident[:])
                p2 = ps_s.tile([H, 128], F32, tag="pss")
                nc.tensor.transpose(p2[:, 0:74], n2[0:74, :],
                                    ident[0:74, 0:74])
                xb = wk.tile([H, 202], F32, tag="xb")
                nc.scalar.copy(xb[:, 0:128], p1[:])
                nc.scalar.copy(xb[:, 128:202], p2[:, 0:74])
                nc.vector.tensor_copy(xn[:, g * NA:g * NA + D], xb[:, 0:D])
                nc.vector.tensor_tensor(xn[:, g * NA + D:g * NA + NA],
                                        xb[:, D:NA], xb[:, NA:202], op=OP.add)
            return xn

        xT = {"d": xT0, "r": xT0}
        for l in range(L):
            ud = conv_units("d", l, xT["d"])
            ur = conv_units("r", l, xT["r"])
            for a, b in zip(ud, ur):
                a(); b()
            for c in "dr":
                xT[c] = assemble(c, l)

        # ---------------- FF head ----------------
        zt = {}
        st2 = ctile([128, 8], F32, "st2")
        nc.vector.memset(st2[:], 0.0)
        for ci, c in enumerate("dr"):
            ps1 = ps_s.tile([H, BL * NA], F32, tag="pss")
            nc.tensor.matmul(ps1[:], ffw1[:], xT[c][:], start=True, stop=True)
            r = wk.tile([H, BL * NA], F32, tag="ffr")
            nc.scalar.activation(r[:], ps1[:], AF.Relu, bias=ffb1c[:])
            ps2 = ps_s.tile([H, BL * NA], F32, tag="pss")
            nc.tensor.matmul(ps2[:], ffw2[:], r[:], start=True, stop=True)
            z = xpool.tile([H, BL * NA], F32, tag="zt")
            nc.vector.scalar_tensor_tensor(z[:], ps2[:], ffb2c[:], xT[c][:],
                                           op0=OP.add, op1=OP.add)
            zt[c] = z
            nc.vector.tensor_reduce(st2[:, 2 * ci:2 * ci + 1], z[:],
                                    axis=AX.X, op=OP.add)
            sq = fpool.tile([H, BL * NA], F16, tag="sq")
            nc.scalar.activation(sq[:], z[:], AF.Square,
                                 accum_out=st2[:, 2 * ci + 1:2 * ci + 2])
        nc.sync.dma_start(ar2_i[:], st2[:])
        if emulate_collectives:
            nc.sync.dma_start(ar2_o[:], ar2_i[:])
        else:
            nc.gpsimd.collective_compute("AllReduce", OP.add, replica_groups=GRP,
                                         ins=[ar2_i], outs=[ar2_o])
        st2o = ctile([128, 8], F32, "st2o")
        nc.sync.dma_start(st2o[:], ar2_o[:])
        for ci, c in enumerate("dr"):
            sc, sh = bn_vecs(st2o, 2 * ci, B * NA, bngc, bnbc, H, f"ff{ci}")
            oT = wk.tile([H, BL * NA], F32, tag="oT")
            nc.vector.tensor_scalar(oT[:], zt[c][:], sc[:], sh[:],
                                    op0=OP.mult, op1=OP.add)
            for g in range(BL):
                pso = ps_s.tile([NA, H], F32, tag="pss")
                nc.tensor.transpose(pso[:], oT[:, g * NA:(g + 1) * NA],
                                    ident[:])
                on = wk.tile([NA, H], F32, tag="on")
                nc.scalar.copy(on[:], pso[:])
                nc.sync.dma_start(o_out[c][g], on[:])

    nc.compile()
    return nc


def _prep_core(inputs, c):
    sl = slice(2 * c, 2 * c + 2)
    x = np.asarray(inputs["x"])[sl]
    dem = np.asarray(inputs["demand"])[sl]
    tw = np.asarray(inputs["time_window"])[sl]
    ds = np.concatenate([x, dem, tw], -1).astype(np.float32)
    dsT = np.ascontiguousarray(ds.transpose(2, 0, 1).reshape(5, BL * NA))
    pkin = np.concatenate([ds[:, D:D + N2], ds[:, D + N2:NA]], -1)
    pkinT = np.ascontiguousarray(pkin.transpose(2, 0, 1).reshape(10, BL * N2))
    dep_nat = np.ones((BL * D, 6), np.float32)
    dep_nat[:, :5] = ds[:, :D].reshape(BL * D, 5)
    pk_nat = np.ones((BL * N2, 11), np.float32)
    pk_nat[:, :10] = pkin.reshape(BL * N2, 10)
    dl_nat = np.ones((BL * N2, 6), np.float32)
    dl_nat[:, :5] = ds[:, D + N2:NA].reshape(BL * N2, 5)
    im = dict(dsT=dsT, pkinT=pkinT, dep_nat=dep_nat, pk_nat=pk_nat,
              dl_nat=dl_nat)
    for c2, key_e, key_m in (("d", "edge_attr_d", "mask_adjacency_d"),
                             ("r", "edge_attr_r", "mask_adjacency_r")):
        ea = np.asarray(inputs[key_e])[sl].reshape(BL, NA, NA, 2)
        im[f"eT_{c2}"] = np.ascontiguousarray(
            ea.transpose(3, 0, 2, 1).reshape(2, COLS)).astype(np.float16)
        tmp = np.zeros((163 * 128, 3), np.float32)
        tmp[:BL * NA * NA, :2] = ea.reshape(BL * NA * NA, 2)
        tmp[:BL * NA * NA, 2] = 1.0
        im[f"e_nat_{c2}"] = np.ascontiguousarray(
            tmp.reshape(163, 128, 3).transpose(1, 0, 2).reshape(128, 489))
        mm = np.asarray(inputs[key_m])[sl].reshape(BL, NA, NA)
        im[f"m_{c2}"] = np.ascontiguousarray(
            mm.transpose(0, 2, 1).reshape(BL, NA * NA)).astype(np.float16)
    for k in ("W0", "W1", "W2", "W3", "W4", "ff_w1", "ff_b1", "ff_w2",
              "ff_b2", "bn_g", "bn_b", "Wvla", "Wvlp", "Wvld",
              "Wga", "Wgp", "Wgd"):
        im[k] = np.asarray(inputs[k], np.float32)
    for i in range(5):
        im[f"b{i}_g"] = np.asarray(inputs[f"b{i}_g"], np.float32)
        im[f"b{i}_b"] = np.asarray(inputs[f"b{i}_b"], np.float32)
    return im


def get_in_maps(inputs):
    return [_prep_core(inputs, c) for c in range(NCORE)]


def kernel(**inputs):
    if "nc" not in _CACHE:
        _CACHE["nc"] = build()
    nc = _CACHE["nc"]
    from concourse.bass_utils import run_bass_kernel_spmd
    in_maps = get_in_maps(inputs)
    res = run_bass_kernel_spmd(nc, in_maps, list(range(NCORE))).results
    od = np.concatenate([res[c]["o_d"] for c in range(NCORE)], 0)
    orr = np.concatenate([res[c]["o_r"] for c in range(NCORE)], 0)
    return od, orr



# revision 2
# speedup vs baseline: 1.0559x; 1.0559x over previous
"""Trainium2 Bass kernel for nn_Encoder (GNN message passing, PDP-VRP encoder).

Sharding: 2 graphs per core x 8 cores. Cross-graph row scramble handled with a
fp16 ReduceScatter in global-flat row order; BatchNorm stats via moment-matrix
AllReduce. Conv compute in feature-major layout:
  psum[h, (j,i)] = la~.T @ E_aug (65 rows: 64 z + mask) + wi.T@x (bcast i)
                 + wj.T@x (bcast j);  BN-shift folded into the Prelu bias.
  P = exp(prelu(psum + bias)) fp16; D = sum_i P; N = sum_i P*x_i; out = N/D.
"""
import numpy as np

B, D, NN = 16, 2, 100
N2, NA = 50, 102
H, HE, L = 128, 64, 3
SLOPE, EPS = 0.2, 1e-5
NCORE = 8
BL = 2                     # graphs per core
COLS = BL * NA * NA        # 20808 edge cols per chain per core
FLAT = B * (NA + 2 * N2)   # 3232 global flat rows
WIN = FLAT // NCORE        # 404 rows per core window
NROW = 65                  # E rows: 64 z + 1 mask

# packed-constant column maps
_CW = 1680
_CWH = 3712
_IW = 1320

_CACHE = {}


def _chunks_full():
    return [(j, 5) for j in range(0, 100, 5)] + [(100, 2)]


def _chunks_sub():
    return [(j, 10) for j in range(0, 50, 10)]


def _groups(chunks, n=3):
    return [chunks[i:i + n] for i in range(0, len(chunks), n)]


def build(gpsimd_offload=True, emulate_collectives=False):
    import concourse.bass as bass
    import concourse.bacc as bacc
    import concourse.tile as tile
    import concourse.mybir as mybir
    from concourse import masks

    dt = mybir.dt
    F32, F16 = dt.float32, dt.float16
    AF = mybir.ActivationFunctionType
    OP = mybir.AluOpType
    AX = mybir.AxisListType

    nc = bacc.Bacc("TRN2", target_bir_lowering=False, debug=False,
                   num_devices=NCORE)

    def din(name, shape, d=F32):
        return nc.dram_tensor(name, shape, d, kind="ExternalInput").ap()

    consts_d = din("consts", [128, _CW])
    consth_d = din("consth", [128, _CWH], F16)
    inputs_d = din("inputs", [128, _IW])
    eT4 = din("eT4", [4, COLS], F16)
    m_in = {c: din(f"m_{c}", [BL * NA * NA], F16) for c in "dr"}

    o_out = {c: nc.dram_tensor(f"o_{c}", [BL, NA, H], F32,
                               kind="ExternalOutput").ap() for c in "dr"}

    E_st = {c: nc.dram_tensor(f"E_{c}", [NROW, BL, NA, NA], F16).ap()
            for c in "dr"}
    rs_in = {c: nc.dram_tensor(f"rsi_{c}", [FLAT * H], F16).ap() for c in "dr"}
    rs_out = {(c, l): nc.dram_tensor(f"rso_{c}{l}", [WIN * H], F16).ap()
              for c in "dr" for l in range(L)}
    mom_scr = nc.dram_tensor("mom_scr", [2, 6], F32).ap()
    ar1_i = nc.dram_tensor("ar1_i", [128, 16], F32).ap()
    ar1_o = nc.dram_tensor("ar1_o", [128, 16], F32).ap()
    ar2_i = {c: nc.dram_tensor(f"ar2_i{c}", [128, 4], F32).ap() for c in "dr"}
    ar2_o = {c: nc.dram_tensor(f"ar2_o{c}", [128, 4], F32).ap() for c in "dr"}
    GRP = [list(range(NCORE))]

    KIDX = {"a": 0, "p": 1, "d": 2}

    def wvcol(k, l):
        return (KIDX[k] * 3 + l) * 128

    import contextlib
    with tile.TileContext(nc) as tc, contextlib.ExitStack() as ctx:
        cpool = ctx.enter_context(tc.tile_pool(name="const", bufs=1))
        wk = ctx.enter_context(tc.tile_pool(name="work", bufs=3))
        xpool = ctx.enter_context(tc.tile_pool(name="xt", bufs=3))
        epool = ctx.enter_context(tc.tile_pool(name="eg", bufs=2))
        fpool = ctx.enter_context(tc.tile_pool(name="f16", bufs=2))
        ps_b = ctx.enter_context(tc.tile_pool(name="psb", bufs=2, space="PSUM"))
        ps_s = ctx.enter_context(tc.tile_pool(name="pss", bufs=2, space="PSUM"))

        def ctile(shape, d, tag):
            return cpool.tile(shape, d, tag=tag, name=tag)

        # ---------- A1/A2: bulk loads + zero-init ----------
        CT = ctile([128, _CW], F32, "consts")
        nc.gpsimd.dma_start(CT[:], consts_d[:])
        CTH = ctile([128, _CWH], F16, "consth")
        nc.gpsimd.dma_start(CTH[:], consth_d[:])
        IT = ctile([128, _IW], F32, "inputs")
        nc.gpsimd.dma_start(IT[:], inputs_d[:])
        eRows = {"d": eT4[0:2, :], "r": eT4[2:4, :]}
        for c in "dr":
            nc.sync.dma_start(
                E_st[c][64:65].rearrange("r b j i -> r (b j i)"),
                m_in[c].unsqueeze(0))

        zsrc = ctile([128, 808], F16, "zsrc")
        nc.vector.memset(zsrc[:], 0.0)
        QE = 128 * 808
        for c in "dr":
            for q in range(4):
                nc.sync.dma_start(
                    rs_in[c][q * QE:(q + 1) * QE].rearrange(
                        "(p a) -> p a", p=128), zsrc[:])

        ident = ctile([128, 128], F32, "ident")
        masks.make_identity(nc, ident[:])
        identh = ctile([128, 128], F16, "identh")
        nc.vector.tensor_copy(identh[:], ident[:])

        # const slices
        Wsl = {
            "W0": CT[0:5, 1152:1280], "W1": CT[0:10, 1280:1408],
            "W2": CT[0:5, 1408:1536],
            "W3": CT[0:2, 1536:1600], "W4": CT[0:2, 1600:1664],
        }
        ffw1 = CTH[:, 3456:3584]
        ffw2 = CTH[:, 3584:3712]
        bcol = {}
        for i, nmv in enumerate(["b0_g", "b0_b", "b1_g", "b1_b", "b2_g",
                                 "b2_b", "b3_g", "b3_b", "b4_g", "b4_b",
                                 "ff_b1", "ff_b2", "bn_g", "bn_b"]):
            hh = 64 if nmv[1] in "34" else 128
            bcol[nmv] = CT[0:hh, 1664 + i:1665 + i]

        W3h = ctile([2, HE], F16, "W3h")
        nc.vector.tensor_copy(W3h[:], Wsl["W3"])
        W4h = ctile([2, HE], F16, "W4h")
        nc.vector.tensor_copy(W4h[:], Wsl["W4"])
        Wesh = {"d": W3h, "r": W4h}
        Wes = {"d": Wsl["W3"], "r": Wsl["W4"]}

        wv = {(k, l): CTH[:, wvcol(k, l):wvcol(k, l) + 128]
              for k in "apd" for l in range(L)}
        wi_s, wj_s = {}, {}
        for k in "apd":
            for l in range(L):
                ti = ctile([H, H], dt.float32r, f"wir{k}{l}")
                nc.vector.tensor_copy(ti[:], CTH[:, 1152 + wvcol(k, l):
                                                 1152 + wvcol(k, l) + 128])
                wi_s[(k, l)] = ti
                tj = ctile([H, H], dt.float32r, f"wjr{k}{l}")
                nc.vector.tensor_copy(tj[:], CTH[:, 2304 + wvcol(k, l):
                                                 2304 + wvcol(k, l) + 128])
                wj_s[(k, l)] = tj

        def wesl(k, l):
            return CT[0:64, wvcol(k, l):wvcol(k, l) + 128]

        ones_col = ctile([128, 1], F32, "ones_col")
        nc.vector.memset(ones_col[:], 1.0)

        with tc.tile_critical():
            pid = nc.scalar.partition_id()

        # ---------- A4: edge embedding into E_st ----------
        def embed_iters(c, act_mod):
            dst = E_st[c].rearrange("r b j i -> r (b j i)")
            iters = []
            it = 0
            for c0 in range(0, COLS, 3072):
                def one(c0=c0, it=it):
                    wA = min(1536, COLS - c0)
                    wB = min(1536, max(0, COLS - c0 - 1536))
                    wT = wA + wB
                    et = fpool.tile([2, 3072], F16, tag="etch", bufs=6)
                    nc.gpsimd.dma_start(et[:, 0:wT], eRows[c][:, c0:c0 + wT])
                    psg = ps_b.tile([128, 1536], F32, tag="psg")
                    for kk in range(3):
                        w = min(512, wA - kk * 512)
                        if w > 0:
                            nc.tensor.matmul(
                                psg[0:64, kk * 512:kk * 512 + w], Wesh[c][:],
                                et[:, kk * 512:kk * 512 + w],
                                start=True, stop=True)
                    for kk in range(3):
                        w = min(512, wB - kk * 512)
                        if w > 0:
                            nc.tensor.matmul(
                                psg[64:128, kk * 512:kk * 512 + w],
                                Wesh[c][:],
                                et[:, 1536 + kk * 512:1536 + kk * 512 + w],
                                start=True, stop=True)
                    ez = fpool.tile([128, 1536], F16, tag="wg", bufs=2)
                    e_ = act_mod[it % len(act_mod)]
                    if e_ == "A":
                        nc.scalar.copy(ez[:], psg[:])
                    elif e_ == "D":
                        nc.vector.tensor_copy(ez[:], psg[:])
                    else:
                        nc.gpsimd.tensor_copy(ez[:], psg[:])
                    nc.scalar.dma_start(dst[0:64, c0:c0 + wA], ez[0:64, 0:wA])
                    if wB > 0:
                        nc.scalar.dma_start(
                            dst[0:64, c0 + 1536:c0 + 1536 + wB],
                            ez[64:128, 0:wB])
                iters.append(one)
                it += 1
            return iters

        # ---------- A5/A6: moments + stats ----------
        stats = ctile([128, 16], F32, "stats")
        nc.vector.memset(stats[:], 0.0)

        def evac(ps_ap, hh, wid, tag, d=F32):
            t = wk.tile([hh, wid], d, tag=tag)
            nc.scalar.copy(t[:], ps_ap)
            return t

        def moments(nat_ap, fdim):
            ps = ps_s.tile([fdim, fdim + 1], F32, tag="pss")
            nc.tensor.matmul(ps[:], nat_ap[:, 0:fdim], nat_ap[:],
                             start=True, stop=True)
            return evac(ps[:], fdim, fdim + 1, f"mom{fdim}")

        def w_transpose(w, kdim, hh):
            ps = ps_s.tile([hh, kdim], F32, tag="pss")
            nc.tensor.transpose(ps[:], w, ident[:kdim, :kdim])
            return evac(ps[:], hh, kdim, "wT")

        def embed_stats(mom, Ws, kdim, hh, scol):
            ps = ps_s.tile([hh, 1], F32, tag="pss")
            nc.tensor.matmul(ps[:], Ws, mom[:, kdim:kdim + 1],
                             start=True, stop=True)
            nc.vector.tensor_copy(stats[0:hh, scol:scol + 1], ps[:])
            ps2 = ps_s.tile([hh, kdim], F32, tag="pss")
            nc.tensor.matmul(ps2[:], Ws, mom[:, 0:kdim],
                             start=True, stop=True)
            G2 = evac(ps2[:], hh, kdim, "G2")
            WT = w_transpose(Ws, kdim, hh)
            prod = wk.tile([hh, kdim], F32, tag="prod")
            nc.vector.tensor_tensor(prod[:], G2[:], WT[:], op=OP.mult)
            nc.vector.tensor_reduce(stats[0:hh, scol + 1:scol + 2], prod[:],
                                    axis=AX.X, op=OP.add)

        embed_stats(moments(IT[0:4, 304:310], 5), Wsl["W0"], 5, H, 0)
        embed_stats(moments(IT[0:100, 312:323], 10), Wsl["W1"], 10, H, 2)
        embed_stats(moments(IT[0:100, 324:330], 5), Wsl["W2"], 5, H, 4)

        # edge raw moments via DVE/ACT stats (layout [128, 163, 3])
        for ci, c in enumerate("dr"):
            c0 = 332 if c == "d" else 824
            nav = IT[:, c0:c0 + 489].rearrange("p (n c) -> p n c", n=163)
            z0, z1 = nav[:, :, 0], nav[:, :, 1]
            st6 = wk.tile([128, 6], F32, tag="st6")
            dummy = wk.tile([128, 163], F32, tag="edum")
            nc.scalar.activation(dummy[:], z0, AF.Square,
                                 accum_out=st6[:, 0:1])
            tmp = wk.tile([128, 163], F32, tag="etmp")
            nc.vector.tensor_tensor(tmp[:], z0, z1, op=OP.mult)
            nc.vector.tensor_reduce(st6[:, 1:2].unsqueeze(2),
                                    tmp[:].unsqueeze(1), axis=AX.X, op=OP.add)
            nc.vector.tensor_reduce(st6[:, 2:3].unsqueeze(2),
                                    z0.unsqueeze(1), axis=AX.X, op=OP.add)
            nc.vector.tensor_copy(st6[:, 3:4], st6[:, 1:2])
            nc.scalar.activation(dummy[:], z1, AF.Square,
                                 accum_out=st6[:, 4:5])
            nc.vector.tensor_reduce(st6[:, 5:6].unsqueeze(2),
                                    z1.unsqueeze(1), axis=AX.X, op=OP.add)
            ps6 = ps_s.tile([1, 6], F32, tag="pss")
            nc.tensor.matmul(ps6[:], ones_col[:], st6[:],
                             start=True, stop=True)
            m6 = evac(ps6[:], 1, 6, "m6")
            nc.sync.dma_start(mom_scr[ci:ci + 1, :], m6[:])
            mom23 = wk.tile([2, 3], F32, tag="mom23")
            nc.sync.dma_start(mom23[:],
                              mom_scr[ci, :].rearrange("(a b) -> a b", a=2))
            embed_stats(mom23, Wes[c], 2, HE, 6 + 2 * ci)

        nc.sync.dma_start(ar1_i[:], stats[:])
        if emulate_collectives:
            nc.sync.dma_start(ar1_o[:], ar1_i[:])
        else:
            nc.gpsimd.collective_compute("AllReduce", OP.add,
                                         replica_groups=GRP,
                                         ins=[ar1_i], outs=[ar1_o])
        sts = ctile([128, 16], F32, "sts")
        nc.sync.dma_start(sts[:], ar1_o[:])

        def bn_vecs(src, scol, n, gc, bc, hh, tag):
            inv = 1.0 / n
            m = wk.tile([hh, 1], F32, tag=f"m{tag}")
            nc.vector.tensor_scalar_mul(m[:], src[0:hh, scol:scol + 1], inv)
            v = wk.tile([hh, 1], F32, tag=f"v{tag}")
            nc.vector.tensor_scalar_mul(v[:], src[0:hh, scol + 1:scol + 2],
                                        inv)
            msq = wk.tile([hh, 1], F32, tag=f"q{tag}")
            nc.vector.tensor_tensor(msq[:], m[:], m[:], op=OP.mult)
            nc.vector.tensor_tensor(v[:], v[:], msq[:], op=OP.subtract)
            nc.vector.tensor_scalar_add(v[:], v[:], EPS)
            sd = wk.tile([hh, 1], F32, tag=f"s{tag}")
            nc.scalar.activation(sd[:], v[:], AF.Sqrt)
            rsd = wk.tile([hh, 1], F32, tag=f"r{tag}")
            nc.vector.reciprocal(rsd[:], sd[:])
            sc = ctile([hh, 1], F32, f"sc{tag}")
            nc.vector.tensor_tensor(sc[:], rsd[:], gc, op=OP.mult)
            sh = ctile([hh, 1], F32, f"sh{tag}")
            nc.vector.tensor_tensor(sh[:], m[:], sc[:], op=OP.mult)
            nc.vector.tensor_tensor(sh[:], bc, sh[:], op=OP.subtract)
            return sc, sh

        sc0, sh0 = bn_vecs(sts, 0, B * D, bcol["b0_g"], bcol["b0_b"], H, "b0")
        sc1, sh1 = bn_vecs(sts, 2, B * N2, bcol["b1_g"], bcol["b1_b"], H, "b1")
        sc2, sh2 = bn_vecs(sts, 4, B * N2, bcol["b2_g"], bcol["b2_b"], H, "b2")
        sce, she = {}, {}
        sce["d"], she["d"] = bn_vecs(sts, 6, B * NA * NA, bcol["b3_g"],
                                     bcol["b3_b"], HE, "b3")
        sce["r"], she["r"] = bn_vecs(sts, 8, B * NA * NA, bcol["b4_g"],
                                     bcol["b4_b"], HE, "b4")

        # ---------- A10: lhsT_aug (65 rows) + prelu bias columns ----------
        laug, biasc = {}, {}
        for c in "dr":
            t = ctile([NROW, 9 * 128], F16, f"laug{c}")
            for k in "apd":
                for l in range(L):
                    w0 = wvcol(k, l)
                    nc.vector.tensor_scalar(t[0:64, w0:w0 + 128],
                                            wesl(k, l), sce[c][:], None,
                                            op0=OP.mult)
            nc.vector.memset(t[64:65, :], 200.0)
            laug[c] = t
            psb = ps_s.tile([128, 9], F32, tag="pss")
            for k in "apd":
                for l in range(L):
                    i = KIDX[k] * 3 + l
                    nc.tensor.matmul(psb[:, i:i + 1], wesl(k, l), she[c][:],
                                     start=True, stop=True)
            bt = ctile([128, 9], F32, f"biasc{c}")
            nc.vector.tensor_scalar_add(bt[:], psb[:], -200.0)
            biasc[c] = bt

        # ---------- A9: node embeddings -> xT0 ----------
        xT0 = xpool.tile([H, BL * NA], F16, tag="xT")
        dsv = IT[0:5, 0:204].rearrange("p (g n) -> p g n", g=BL)
        x0v = xT0[:].rearrange("p (g n) -> p g n", g=BL)
        ps = ps_s.tile([H, BL * D], F32, tag="pss")
        nc.tensor.matmul(ps[:], Wsl["W0"], dsv[:, :, 0:D], start=True,
                         stop=True)
        nc.vector.tensor_scalar(
            x0v[:, :, 0:D], ps[:].rearrange("p (g n) -> p g n", g=BL),
            sc0[:], sh0[:], op0=OP.mult, op1=OP.add)
        ps = ps_s.tile([H, BL * N2], F32, tag="pss")
        nc.tensor.matmul(ps[:], Wsl["W1"],
                         IT[0:10, 204:304].rearrange("p (g n) -> p g n", g=BL),
                         start=True, stop=True)
        nc.vector.tensor_scalar(
            x0v[:, :, D:D + N2], ps[:].rearrange("p (g n) -> p g n", g=BL),
            sc1[:], sh1[:], op0=OP.mult, op1=OP.add)
        ps = ps_s.tile([H, BL * N2], F32, tag="pss")
        nc.tensor.matmul(ps[:], Wsl["W2"], dsv[:, :, D + N2:NA],
                         start=True, stop=True)
        nc.vector.tensor_scalar(
            x0v[:, :, D + N2:NA], ps[:].rearrange("p (g n) -> p g n", g=BL),
            sc2[:], sh2[:], op0=OP.mult, op1=OP.add)

        for f_ in embed_iters("d", "AD"):
            f_()
        for f_ in embed_iters("r", "AD"):
            f_()

        def ghook_fn():
            pass

        # ---------- conv layers ----------
        ucount = [0]

        def mm_evac(w, rhs_ap, wid, tag):
            ps = ps_s.tile([H, wid], F32, tag="pss")
            nc.tensor.matmul(ps[:], w, rhs_ap, start=True, stop=True)
            t = xpool.tile([H, wid], dt.float32r, tag=tag)
            nc.scalar.copy(t[:], ps[:])
            th = xpool.tile([H, wid], F16, tag=tag + "h")
            nc.vector.tensor_copy(th[:], t[:])
            return t, th

        def prep_chain(c, l, xTin):
            xv = xTin[:].rearrange("p (g n) -> p g n", g=BL)
            xall, xallh = mm_evac(wv[("a", l)], xTin[:], BL * NA, "xa")
            pick, pickh = mm_evac(wv[("p", l)], xv[:, :, D:D + N2],
                                  BL * N2, "xp")
            deli, delih = mm_evac(wv[("d", l)], xv[:, :, D + N2:NA],
                                  BL * N2, "xd")
            return [("a", xall, xallh, NA, _chunks_full()),
                    ("p", pick, pickh, N2, _chunks_sub()),
                    ("d", deli, delih, N2, _chunks_sub())]

        def fetch_eg(c, k, g, S):
            Eg = epool.tile([NROW, S * S], F16, tag=f"Eg{k}", bufs=2)
            if k == "a":
                half = (S // 2) * S
                nc.sync.dma_start(Eg[:, 0:half],
                                  E_st[c][:, g, 0:S // 2, :])
                nc.sync.dma_start(Eg[:, half:S * S],
                                  E_st[c][:, g, S // 2:S, :])
            elif k == "p":
                nc.sync.dma_start(Eg[:, 0:S * S],
                                  E_st[c][:, g, D:D + N2, D:D + N2])
            else:
                nc.sync.dma_start(Eg[:, 0:S * S],
                                  E_st[c][:, g, D + N2:NA, D + N2:NA])
            return Eg

        def emit_units(c, l, cfg, hooks, eg0=None):
            i = 0
            for k_, xk_, xkh_, S_, chunks_ in cfg:
                for g_ in range(BL):
                    emit_unit(c, l, k_, xk_, xkh_, S_, chunks_, g_,
                              eg0 if i == 0 else None)
                    if i in hooks:
                        hooks[i]()
                    i += 1

        flushq = []
        ghook = [None]

        def drainq(keep=0):
            while len(flushq) > keep:
                flushq.pop(0)()

        def emit_unit(c, l, k, xk, xkh, S, chunks, g, eg=None):
            ilen = S
            uc = ucount[0]
            ucount[0] += 1
            la = laug[c][:, wvcol(k, l):wvcol(k, l) + 128]
            bias_ap = biasc[c][:, KIDX[k] * 3 + l:KIDX[k] * 3 + l + 1]
            wi, wjt = wi_s[(k, l)], wj_s[(k, l)]
            Eg = eg if eg is not None else fetch_eg(c, k, g, S)

            Dt = wk.tile([H, S], F16, tag="Dt")
            Nt = wk.tile([H, S], F16, tag="Nt")
            grps = _groups(chunks)
            xkb = xkh[:, g * S:g * S + ilen]
            hl = ilen // 2
            pend = []
            tgp = None

            def make_flush(Pgt, mypend):
                def fl():
                    for half, grp, fd, gix in mypend:
                        base_off = half * 1536
                        runs = []
                        for ki, (j0, nj) in enumerate(grp):
                            if runs and runs[-1][2] == nj:
                                runs[-1][1] += 1
                            else:
                                runs.append([ki, 1, nj, j0])
                        wg = fpool.tile([128, 1536], F16, tag="wg", bufs=2)
                        pf = fpool.tile([128, 768], F16, tag="pf", bufs=2)
                        wf = fpool.tile([128, 768], F16, tag="wf", bufs=2)
                        for k0, nk, nj, j0r in runs:
                            pbase = Pgt[:, base_off + k0 * 512:
                                        base_off + (k0 + nk) * 512]
                            wbase = wg[:, k0 * 512:(k0 + nk) * 512]
                            pv4 = pbase.rearrange("p (k r) -> p k r", k=nk)\
                                [:, :, 0:nj * ilen].rearrange(
                                "p k (j i) -> p k j i", i=ilen)
                            wv4 = wbase.rearrange("p (k r) -> p k r", k=nk)\
                                [:, :, 0:nj * ilen].rearrange(
                                "p k (j i) -> p k j i", i=ilen)
                            xb4 = xkb.unsqueeze(1).unsqueeze(1)\
                                .broadcast_to([H, nk, nj, ilen])
                            nc.vector.tensor_tensor(wv4, pv4, xb4, op=OP.mult)
                            pf4 = pf[:, k0 * 256:k0 * 256 + nk * nj * hl]\
                                .rearrange("p (k j i) -> p k j i", k=nk, j=nj)
                            wf4 = wf[:, k0 * 256:k0 * 256 + nk * nj * hl]\
                                .rearrange("p (k j i) -> p k j i", k=nk, j=nj)
                            nc.gpsimd.tensor_tensor(
                                pf4, pv4[:, :, :, 0:hl],
                                pv4[:, :, :, hl:2 * hl], op=OP.add)
                            wf_pool = (k == "a") and (gix % 2 == 0)
                            eng = nc.gpsimd if wf_pool else nc.vector
                            eng.tensor_tensor(
                                wf4, wv4[:, :, :, 0:hl],
                                wv4[:, :, :, hl:2 * hl], op=OP.add)
                            dv = Dt[:, j0r:j0r + nk * nj].rearrange(
                                "p (k j) -> p k j", k=nk)
                            nv = Nt[:, j0r:j0r + nk * nj].rearrange(
                                "p (k j) -> p k j", k=nk)
                            with nc.allow_low_precision("fp16 softmax sums"):
                                nc.vector.tensor_reduce(dv, pf4, axis=AX.X,
                                                        op=OP.add)
                                nc.vector.tensor_reduce(nv, wf4, axis=AX.X,
                                                        op=OP.add)
                return fl

            n_grps = len(grps)
            for gix, grp in enumerate(grps):
                half = gix % 2
                if half == 0:
                    tgp = fpool.tile([128, 3072], F16, tag="tg", bufs=2)
                j0g = grp[0][0]
                psg = ps_b.tile([128, 1536], F32, tag="psg")
                eoff = j0g * ilen
                for ki, (j0, nj) in enumerate(grp):
                    nc.tensor.matmul(
                        psg[:, ki * 512:ki * 512 + nj * ilen],
                        la, Eg[:, eoff:eoff + nj * ilen],
                        start=True, stop=False)
                    eoff += nj * ilen
                for ki, (j0, nj) in enumerate(grp):
                    a_rhs = xk[:, g * S + j0:g * S + j0 + nj]\
                        .unsqueeze(2).broadcast_to([H, nj, ilen])
                    nc.tensor.matmul(
                        psg[:, ki * 512:ki * 512 + nj * ilen],
                        wi[:], a_rhs, start=False, stop=False)
                b_base = xk[:, g * S:g * S + ilen]
                for ki, (j0, nj) in enumerate(grp):
                    b_rhs = b_base.unsqueeze(1).broadcast_to([H, nj, ilen])
                    nc.tensor.matmul(
                        psg[:, ki * 512:ki * 512 + nj * ilen],
                        wjt[:], b_rhs, start=False, stop=True)
                fd = (len(grp) - 1) * 512 + grp[-1][1] * ilen
                nc.scalar.activation(tgp[:, half * 1536:half * 1536 + fd],
                                     psg[:, 0:fd], AF.Prelu, alpha=SLOPE,
                                     bias=bias_ap)
                pend.append((half, grp, fd, gix))
                if ghook[0] is not None:
                    ghook[0]()
                if half == 1 or gix == n_grps - 1:
                    span = half * 1536 + fd
                    Pgp = fpool.tile([128, 3072], F16, tag="Pg", bufs=3)
                    nc.scalar.activation(Pgp[:, 0:span], tgp[:, 0:span],
                                         AF.Exp)
                    flushq.append(make_flush(Pgp, list(pend)))
                    pend.clear()
                    drainq(1)

            def tail():
                Di = wk.tile([H, S], F32, tag="Di")
                nc.vector.reciprocal(Di[:], Dt[:])
                og = wk.tile([H, S], F16, tag="og")
                nc.vector.tensor_tensor(og[:], Nt[:], Di[:], op=OP.mult)
                pst = ps_s.tile([S, H], F16, tag="pss")
                nc.tensor.transpose(pst[:], og[:], identh[:])
                nat2 = wk.tile([S, H], F16, tag="nat2")
                if uc % 2 == 0:
                    nc.scalar.copy(nat2[:], pst[:])
                else:
                    nc.vector.tensor_copy(nat2[:], pst[:])
                if k == "a":
                    off = pid * (BL * NA * H) + g * NA * H
                elif k == "p":
                    off = pid * (BL * N2 * H) + g * N2 * H + B * NA * H
                else:
                    off = pid * (BL * N2 * H) + g * N2 * H \
                        + B * (NA + N2) * H
                nc.scalar.dma_start(
                    rs_in[c][bass.ds(off, S * H)].rearrange(
                        "(r h) -> r h", h=H), nat2[:])

            flushq.append(tail)
            drainq(1)

        def emit_rs(c, l):
            if emulate_collectives:
                nc.sync.dma_start(rs_out[(c, l)][:], rs_in[c][0:WIN * H])
            else:
                with nc.allow_low_precision("fp16 reduce-scatter"):
                    nc.gpsimd.collective_compute(
                        "ReduceScatter", OP.add, replica_groups=GRP,
                        ins=[rs_in[c]], outs=[rs_out[(c, l)]])

        def assemble(c, l):
            rsv = rs_out[(c, l)].rearrange("(r h) -> r h", h=H)
            xn = xpool.tile([H, BL * NA], F16, tag="xT")
            for g in range(BL):
                n1 = wk.tile([128, H], F16, tag="asm")
                nc.sync.dma_start(n1[:], rsv[g * 202:g * 202 + 128, :])
                n2 = wk.tile([128, H], F16, tag="asm")
                nc.sync.dma_start(
                    n2[0:74, :], rsv[g * 202 + 128:g * 202 + 202, :])
                p1 = ps_s.tile([H, 128], F16, tag="pss")
                nc.tensor.transpose(p1[:], n1[:], identh[:])
                p2 = ps_s.tile([H, 128], F16, tag="pss")
                nc.tensor.transpose(p2[:, 0:74], n2[0:74, :],
                                    identh[0:74, 0:74])
                xb = wk.tile([H, 202], F16, tag="xb")
                nc.scalar.copy(xb[:, 0:128], p1[:])
                nc.scalar.copy(xb[:, 128:202], p2[:, 0:74])
                nc.scalar.copy(xn[:, g * NA:g * NA + D], xb[:, 0:D])
                nc.vector.tensor_tensor(xn[:, g * NA + D:g * NA + NA],
                                        xb[:, D:NA], xb[:, NA:202],
                                        op=OP.add)
            return xn

        xT = {"d": xT0, "r": xT0}
        cfgs = {}
        egpre = {}
        cfgs[("d", 0)] = prep_chain("d", 0, xT0)
        egpre[("d", 0)] = fetch_eg("d", "a", 0, NA)

        def unit_closures(c, l):
            out = []
            i = 0
            for k_, xk_, xkh_, S_, chunks_ in cfgs[(c, l)]:
                for g_ in range(BL):
                    def fn(k=k_, xk=xk_, xkh=xkh_, S=S_, ch=chunks_, g=g_,
                           ii=i, cc=c, ll=l):
                        emit_unit(cc, ll, k, xk, xkh, S, ch, g,
                                  egpre.pop((cc, ll), None) if ii == 0
                                  else None)
                    out.append(fn)
                    i += 1
            return out

        st2 = {c: ctile([128, 4], F32, f"st2{c}") for c in "dr"}
        for c_ in "dr":
            nc.vector.memset(st2[c_][:], 0.0)
        zt = {}

        def ff_chain(c, xc):
            ps1 = ps_s.tile([H, BL * NA], F32, tag="pss")
            nc.tensor.matmul(ps1[:], ffw1, xc[:], start=True, stop=True)
            r = wk.tile([H, BL * NA], F16, tag="ffr")
            nc.scalar.activation(r[:], ps1[:], AF.Relu, bias=bcol["ff_b1"])
            ps2 = ps_s.tile([H, BL * NA], F32, tag="pss")
            nc.tensor.matmul(ps2[:], ffw2, r[:], start=True, stop=True)
            z = xpool.tile([H, BL * NA], F32, tag="zt")
            nc.vector.scalar_tensor_tensor(z[:], ps2[:], bcol["ff_b2"],
                                           xc[:], op0=OP.add, op1=OP.add)
            zt[c] = z
            nc.vector.tensor_reduce(st2[c][:, 0:1], z[:],
                                    axis=AX.X, op=OP.add)
            sq = fpool.tile([H, BL * NA], F16, tag="sq")
            nc.scalar.activation(sq[:], z[:], AF.Square,
                                 accum_out=st2[c][:, 1:2])
            nc.sync.dma_start(ar2_i[c][:], st2[c][:])
            if emulate_collectives:
                nc.sync.dma_start(ar2_o[c][:], ar2_i[c][:])
            else:
                nc.gpsimd.collective_compute("AllReduce", OP.add,
                                             replica_groups=GRP,
                                             ins=[ar2_i[c]], outs=[ar2_o[c]])
            st2o = ctile([128, 4], F32, f"st2o{c}")
            nc.sync.dma_start(st2o[:], ar2_o[c][:])
            sc, sh = bn_vecs(st2o, 0, B * NA, bcol["bn_g"],
                             bcol["bn_b"], H, f"ff{c}")
            oT = wk.tile([H, BL * NA], F32, tag="oT")
            nc.vector.tensor_scalar(oT[:], zt[c][:], sc[:], sh[:],
                                    op0=OP.mult, op1=OP.add)
            for g in range(BL):
                pso = ps_s.tile([NA, H], F32, tag="pss")
                nc.tensor.transpose(pso[:], oT[:, g * NA:(g + 1) * NA],
                                    ident[:])
                on = wk.tile([NA, H], F32, tag="on")
                nc.scalar.copy(on[:], pso[:])
                nc.sync.dma_start(o_out[c][g], on[:])

        phases = [(c, l) for l in range(L) for c in "dr"]
        carried = False
        pending_rs = [None]
        for pi, (c, l) in enumerate(phases):
            U = unit_closures(c, l)
            start = 1 if carried else 0
            carried = False
            nxt = phases[pi + 1] if pi + 1 < len(phases) else None
            for ui in range(start, 5):
                U[ui]()
                if ui == 1 and pending_rs[0] is not None:
                    drainq(0)
                    pending_rs[0]()
                    pending_rs[0] = None
                    if nxt is not None and nxt[1] > 0:
                        xT[nxt[0]] = assemble(nxt[0], nxt[1] - 1)
                    if nxt is None:
                        xT["d"] = assemble("d", L - 1)
                if pi == 0 and ui == 1:
                    xT["r"] = xT0
                if nxt is not None and ui == 2:
                    cfgs[nxt] = prep_chain(nxt[0], nxt[1], xT[nxt[0]])
                    egpre[nxt] = fetch_eg(nxt[0], "a", 0, NA)
                if nxt is None and ui == 4:
                    ff_chain("d", xT["d"])
            if nxt is not None:
                unit_closures(*nxt)[0]()
                carried = True
            U[5]()
            pending_rs[0] = (lambda cc=c, ll=l: emit_rs(cc, ll))
        drainq(0)
        pending_rs[0]()

        # ---------- FF head ----------
        xTr = assemble("r", L - 1)
        ff_chain("r", xTr)

    nc.compile()
    return nc


def _prep_core(inputs, core):
    sl = slice(2 * core, 2 * core + 2)
    x = np.asarray(inputs["x"])[sl]
    dem = np.asarray(inputs["demand"])[sl]
    tw = np.asarray(inputs["time_window"])[sl]
    ds = np.concatenate([x, dem, tw], -1).astype(np.float32)
    dsT = np.ascontiguousarray(ds.transpose(2, 0, 1).reshape(5, BL * NA))
    pkin = np.concatenate([ds[:, D:D + N2], ds[:, D + N2:NA]], -1)
    pkinT = np.ascontiguousarray(pkin.transpose(2, 0, 1).reshape(10, BL * N2))

    IT = np.zeros((128, _IW), np.float32)
    IT[0:5, 0:204] = dsT
    IT[0:10, 204:304] = pkinT
    IT[0:4, 304:309] = ds[:, :D].reshape(BL * D, 5)
    IT[0:4, 309] = 1.0
    IT[0:100, 312:322] = pkin.reshape(BL * N2, 10)
    IT[0:100, 322] = 1.0
    IT[0:100, 324:329] = ds[:, D + N2:NA].reshape(BL * N2, 5)
    IT[0:100, 329] = 1.0

    eT4 = np.zeros((4, COLS), np.float16)
    ms = {}
    for ci, (c2, key_e, key_m) in enumerate(
            (("d", "edge_attr_d", "mask_adjacency_d"),
             ("r", "edge_attr_r", "mask_adjacency_r"))):
        ea = np.asarray(inputs[key_e])[sl].reshape(BL, NA, NA, 2)
        eT4[2 * ci:2 * ci + 2] = ea.transpose(3, 0, 2, 1).reshape(2, COLS)
        tmp = np.zeros((163 * 128, 3), np.float32)
        tmp[:BL * NA * NA, :2] = ea.reshape(BL * NA * NA, 2)
        tmp[:BL * NA * NA, 2] = 1.0
        nat = tmp.reshape(163, 128, 3).transpose(1, 0, 2).reshape(128, 489)
        c0 = 332 if c2 == "d" else 824
        IT[:, c0:c0 + 489] = nat
        mm = np.asarray(inputs[key_m])[sl].reshape(BL, NA, NA)
        ms[c2] = np.ascontiguousarray(
            mm.transpose(0, 2, 1).reshape(BL * NA * NA)).astype(np.float16)

    CT = np.zeros((128, _CW), np.float32)
    CTH = np.zeros((128, _CWH), np.float16)
    KI = {"a": 0, "p": 1, "d": 2}
    Wvl = {"a": np.asarray(inputs["Wvla"], np.float32),
           "p": np.asarray(inputs["Wvlp"], np.float32),
           "d": np.asarray(inputs["Wvld"], np.float32)}
    Wgx = {"a": np.asarray(inputs["Wga"], np.float32),
           "p": np.asarray(inputs["Wgp"], np.float32),
           "d": np.asarray(inputs["Wgd"], np.float32)}
    for k in "apd":
        for l in range(L):
            w0 = (KI[k] * 3 + l) * 128
            CTH[:, w0:w0 + 128] = Wvl[k][l]
            CTH[:, 1152 + w0:1152 + w0 + 128] = Wgx[k][l, 0:H, :]
            CTH[:, 2304 + w0:2304 + w0 + 128] = Wgx[k][l, H:2 * H, :]
            CT[0:64, w0:w0 + 128] = Wgx[k][l, 2 * H:2 * H + HE, :]
    CTH[:, 3456:3584] = np.asarray(inputs["ff_w1"], np.float32)
    CTH[:, 3584:3712] = np.asarray(inputs["ff_w2"], np.float32)
    CT[0:5, 1152:1280] = np.asarray(inputs["W0"], np.float32)
    CT[0:10, 1280:1408] = np.asarray(inputs["W1"], np.float32)
    CT[0:5, 1408:1536] = np.asarray(inputs["W2"], np.float32)
    CT[0:2, 1536:1600] = np.asarray(inputs["W3"], np.float32)
    CT[0:2, 1600:1664] = np.asarray(inputs["W4"], np.float32)
    for i, nmv in enumerate(["b0_g", "b0_b", "b1_g", "b1_b", "b2_g", "b2_b",
                             "b3_g", "b3_b", "b4_g", "b4_b",
                             "ff_b1", "ff_b2", "bn_g", "bn_b"]):
        v = np.asarray(inputs[nmv], np.float32)
        CT[0:v.shape[0], 1664 + i] = v

    return dict(consts=CT, consth=CTH, inputs=IT, eT4=eT4,
                m_d=ms["d"], m_r=ms["r"])


def get_in_maps(inputs):
    return [_prep_core(inputs, c) for c in range(NCORE)]


def kernel(**inputs):
    if "nc" not in _CACHE:
        _CACHE["nc"] = build()
    nc = _CACHE["nc"]
    from concourse.bass_utils import run_bass_kernel_spmd
    in_maps = get_in_maps(inputs)
    res = run_bass_kernel_spmd(nc, in_maps, list(range(NCORE))).results
    od = np.concatenate([res[c]["o_d"] for c in range(NCORE)], 0)
    orr = np.concatenate([res[c]["o_r"] for c in range(NCORE)], 0)
    return od, orr


# revision 3
# speedup vs baseline: 1.0575x; 1.0015x over previous
"""Trainium2 Bass kernel for nn_Encoder (GNN message passing, PDP-VRP encoder).

Sharding: 2 graphs per core x 8 cores. Cross-graph row scramble handled with a
fp16 ReduceScatter in global-flat row order; BatchNorm stats via moment-matrix
AllReduce. Conv compute in feature-major layout:
  psum[h, (j,i)] = la~.T @ E_aug (65 rows: 64 z + mask) + wi.T@x (bcast i)
                 + wj.T@x (bcast j);  BN-shift folded into the Prelu bias.
  P = exp(prelu(psum + bias)) fp16; D = sum_i P; N = sum_i P*x_i; out = N/D.
"""
import numpy as np

B, D, NN = 16, 2, 100
N2, NA = 50, 102
H, HE, L = 128, 64, 3
SLOPE, EPS = 0.2, 1e-5
NCORE = 8
BL = 2                     # graphs per core
COLS = BL * NA * NA        # 20808 edge cols per chain per core
FLAT = B * (NA + 2 * N2)   # 3232 global flat rows
WIN = FLAT // NCORE        # 404 rows per core window
NROW = 65                  # E rows: 64 z + 1 mask

# packed-constant column maps
_CW = 1680
_CWH = 3712
_IW = 1320

_CACHE = {}


def _chunks_full():
    return [(j, 5) for j in range(0, 100, 5)] + [(100, 2)]


def _chunks_sub():
    return [(j, 10) for j in range(0, 50, 10)]


def _groups(chunks, n=3):
    return [chunks[i:i + n] for i in range(0, len(chunks), n)]


def build(gpsimd_offload=True, emulate_collectives=False):
    import concourse.bass as bass
    import concourse.bacc as bacc
    import concourse.tile as tile
    import concourse.mybir as mybir
    from concourse import masks

    dt = mybir.dt
    F32, F16 = dt.float32, dt.float16
    AF = mybir.ActivationFunctionType
    OP = mybir.AluOpType
    AX = mybir.AxisListType

    nc = bacc.Bacc("TRN2", target_bir_lowering=False, debug=False,
                   num_devices=NCORE)

    def din(name, shape, d=F32):
        return nc.dram_tensor(name, shape, d, kind="ExternalInput").ap()

    consts_d = din("consts", [128, _CW])
    consth_d = din("consth", [128, _CWH], F16)
    inputs_d = din("inputs", [128, _IW])
    eT4 = din("eT4", [4, COLS], F16)
    m_in = {c: din(f"m_{c}", [BL * NA * NA], F16) for c in "dr"}

    o_out = {c: nc.dram_tensor(f"o_{c}", [BL, NA, H], F32,
                               kind="ExternalOutput").ap() for c in "dr"}

    E_st = {c: nc.dram_tensor(f"E_{c}", [NROW, BL, NA, NA], F16).ap()
            for c in "dr"}
    rs_in = {c: nc.dram_tensor(f"rsi_{c}", [FLAT * H], F16).ap() for c in "dr"}
    rs_out = {(c, l): nc.dram_tensor(f"rso_{c}{l}", [WIN * H], F16).ap()
              for c in "dr" for l in range(L)}
    mom_scr = nc.dram_tensor("mom_scr", [2, 6], F32).ap()
    ar1_i = nc.dram_tensor("ar1_i", [128, 16], F32).ap()
    ar1_o = nc.dram_tensor("ar1_o", [128, 16], F32).ap()
    ar2_i = {c: nc.dram_tensor(f"ar2_i{c}", [128, 4], F32).ap() for c in "dr"}
    ar2_o = {c: nc.dram_tensor(f"ar2_o{c}", [128, 4], F32).ap() for c in "dr"}
    GRP = [list(range(NCORE))]

    KIDX = {"a": 0, "p": 1, "d": 2}

    def wvcol(k, l):
        return (KIDX[k] * 3 + l) * 128

    import contextlib
    with tile.TileContext(nc) as tc, contextlib.ExitStack() as ctx:
        cpool = ctx.enter_context(tc.tile_pool(name="const", bufs=1))
        wk = ctx.enter_context(tc.tile_pool(name="work", bufs=3))
        xpool = ctx.enter_context(tc.tile_pool(name="xt", bufs=3))
        epool = ctx.enter_context(tc.tile_pool(name="eg", bufs=2))
        fpool = ctx.enter_context(tc.tile_pool(name="f16", bufs=2))
        ps_b = ctx.enter_context(tc.tile_pool(name="psb", bufs=2, space="PSUM"))
        ps_s = ctx.enter_context(tc.tile_pool(name="pss", bufs=2, space="PSUM"))

        def ctile(shape, d, tag):
            return cpool.tile(shape, d, tag=tag, name=tag)

        # ---------- A1/A2: bulk loads + zero-init ----------
        CT = ctile([128, _CW], F32, "consts")
        nc.gpsimd.dma_start(CT[:], consts_d[:])
        CTH = ctile([128, _CWH], F16, "consth")
        nc.gpsimd.dma_start(CTH[:], consth_d[:])
        IT = ctile([128, _IW], F32, "inputs")
        nc.gpsimd.dma_start(IT[:], inputs_d[:])
        eRows = {"d": eT4[0:2, :], "r": eT4[2:4, :]}
        for c in "dr":
            nc.sync.dma_start(
                E_st[c][64:65].rearrange("r b j i -> r (b j i)"),
                m_in[c].unsqueeze(0))

        zsrc = ctile([128, 808], F16, "zsrc")
        nc.vector.memset(zsrc[:], 0.0)
        QE = 128 * 808
        for c in "dr":
            for q in range(4):
                nc.sync.dma_start(
                    rs_in[c][q * QE:(q + 1) * QE].rearrange(
                        "(p a) -> p a", p=128), zsrc[:])

        ident = ctile([128, 128], F32, "ident")
        masks.make_identity(nc, ident[:])
        identh = ctile([128, 128], F16, "identh")
        nc.vector.tensor_copy(identh[:], ident[:])

        # const slices
        Wsl = {
            "W0": CT[0:5, 1152:1280], "W1": CT[0:10, 1280:1408],
            "W2": CT[0:5, 1408:1536],
            "W3": CT[0:2, 1536:1600], "W4": CT[0:2, 1600:1664],
        }
        ffw1 = CTH[:, 3456:3584]
        ffw2 = CTH[:, 3584:3712]
        bcol = {}
        for i, nmv in enumerate(["b0_g", "b0_b", "b1_g", "b1_b", "b2_g",
                                 "b2_b", "b3_g", "b3_b", "b4_g", "b4_b",
                                 "ff_b1", "ff_b2", "bn_g", "bn_b"]):
            hh = 64 if nmv[1] in "34" else 128
            bcol[nmv] = CT[0:hh, 1664 + i:1665 + i]

        W3h = ctile([2, HE], F16, "W3h")
        nc.vector.tensor_copy(W3h[:], Wsl["W3"])
        W4h = ctile([2, HE], F16, "W4h")
        nc.vector.tensor_copy(W4h[:], Wsl["W4"])
        Wesh = {"d": W3h, "r": W4h}
        Wes = {"d": Wsl["W3"], "r": Wsl["W4"]}

        wv = {(k, l): CTH[:, wvcol(k, l):wvcol(k, l) + 128]
              for k in "apd" for l in range(L)}
        wi_s, wj_s = {}, {}
        for k in "apd":
            for l in range(L):
                ti = ctile([H, H], dt.float32r, f"wir{k}{l}")
                nc.vector.tensor_copy(ti[:], CTH[:, 1152 + wvcol(k, l):
                                                 1152 + wvcol(k, l) + 128])
                wi_s[(k, l)] = ti
                tj = ctile([H, H], dt.float32r, f"wjr{k}{l}")
                nc.vector.tensor_copy(tj[:], CTH[:, 2304 + wvcol(k, l):
                                                 2304 + wvcol(k, l) + 128])
                wj_s[(k, l)] = tj

        def wesl(k, l):
            return CT[0:64, wvcol(k, l):wvcol(k, l) + 128]

        ones_col = ctile([128, 1], F32, "ones_col")
        nc.vector.memset(ones_col[:], 1.0)

        with tc.tile_critical():
            pid = nc.sync.partition_id()

        # ---------- A4: edge embedding into E_st ----------
        def embed_iters(c, act_mod):
            dst = E_st[c].rearrange("r b j i -> r (b j i)")
            iters = []
            it = 0
            for c0 in range(0, COLS, 3072):
                def one(c0=c0, it=it):
                    wA = min(1536, COLS - c0)
                    wB = min(1536, max(0, COLS - c0 - 1536))
                    wT = wA + wB
                    et = fpool.tile([2, 3072], F16, tag="etch", bufs=6)
                    nc.gpsimd.dma_start(et[:, 0:wT], eRows[c][:, c0:c0 + wT])
                    psg = ps_b.tile([128, 1536], F32, tag="psg")
                    for kk in range(3):
                        w = min(512, wA - kk * 512)
                        if w > 0:
                            nc.tensor.matmul(
                                psg[0:64, kk * 512:kk * 512 + w], Wesh[c][:],
                                et[:, kk * 512:kk * 512 + w],
                                start=True, stop=True)
                    for kk in range(3):
                        w = min(512, wB - kk * 512)
                        if w > 0:
                            nc.tensor.matmul(
                                psg[64:128, kk * 512:kk * 512 + w],
                                Wesh[c][:],
                                et[:, 1536 + kk * 512:1536 + kk * 512 + w],
                                start=True, stop=True)
                    ez = fpool.tile([128, 1536], F16, tag="wg", bufs=2)
                    e_ = act_mod[it % len(act_mod)]
                    if e_ == "A":
                        nc.scalar.copy(ez[:], psg[:])
                    elif e_ == "D":
                        nc.vector.tensor_copy(ez[:], psg[:])
                    else:
                        nc.gpsimd.tensor_copy(ez[:], psg[:])
                    nc.scalar.dma_start(dst[0:64, c0:c0 + wA], ez[0:64, 0:wA])
                    if wB > 0:
                        nc.scalar.dma_start(
                            dst[0:64, c0 + 1536:c0 + 1536 + wB],
                            ez[64:128, 0:wB])
                iters.append(one)
                it += 1
            return iters

        # ---------- A5/A6: moments + stats ----------
        stats = ctile([128, 16], F32, "stats")
        nc.vector.memset(stats[:], 0.0)

        def evac(ps_ap, hh, wid, tag, d=F32):
            t = wk.tile([hh, wid], d, tag=tag)
            nc.scalar.copy(t[:], ps_ap)
            return t

        def moments(nat_ap, fdim):
            ps = ps_s.tile([fdim, fdim + 1], F32, tag="pss")
            nc.tensor.matmul(ps[:], nat_ap[:, 0:fdim], nat_ap[:],
                             start=True, stop=True)
            return evac(ps[:], fdim, fdim + 1, f"mom{fdim}")

        def w_transpose(w, kdim, hh):
            ps = ps_s.tile([hh, kdim], F32, tag="pss")
            nc.tensor.transpose(ps[:], w, ident[:kdim, :kdim])
            return evac(ps[:], hh, kdim, "wT")

        def embed_stats(mom, Ws, kdim, hh, scol):
            ps = ps_s.tile([hh, 1], F32, tag="pss")
            nc.tensor.matmul(ps[:], Ws, mom[:, kdim:kdim + 1],
                             start=True, stop=True)
            nc.vector.tensor_copy(stats[0:hh, scol:scol + 1], ps[:])
            ps2 = ps_s.tile([hh, kdim], F32, tag="pss")
            nc.tensor.matmul(ps2[:], Ws, mom[:, 0:kdim],
                             start=True, stop=True)
            G2 = evac(ps2[:], hh, kdim, "G2")
            WT = w_transpose(Ws, kdim, hh)
            prod = wk.tile([hh, kdim], F32, tag="prod")
            nc.vector.tensor_tensor(prod[:], G2[:], WT[:], op=OP.mult)
            nc.vector.tensor_reduce(stats[0:hh, scol + 1:scol + 2], prod[:],
                                    axis=AX.X, op=OP.add)

        embed_stats(moments(IT[0:4, 304:310], 5), Wsl["W0"], 5, H, 0)
        embed_stats(moments(IT[0:100, 312:323], 10), Wsl["W1"], 10, H, 2)
        embed_stats(moments(IT[0:100, 324:330], 5), Wsl["W2"], 5, H, 4)

        # edge raw moments via DVE/ACT stats (layout [128, 163, 3])
        for ci, c in enumerate("dr"):
            c0 = 332 if c == "d" else 824
            nav = IT[:, c0:c0 + 489].rearrange("p (n c) -> p n c", n=163)
            z0, z1 = nav[:, :, 0], nav[:, :, 1]
            st6 = wk.tile([128, 6], F32, tag="st6")
            dummy = wk.tile([128, 163], F32, tag="edum")
            nc.scalar.activation(dummy[:], z0, AF.Square,
                                 accum_out=st6[:, 0:1])
            tmp = wk.tile([128, 163], F32, tag="etmp")
            nc.vector.tensor_tensor(tmp[:], z0, z1, op=OP.mult)
            nc.vector.tensor_reduce(st6[:, 1:2].unsqueeze(2),
                                    tmp[:].unsqueeze(1), axis=AX.X, op=OP.add)
            nc.vector.tensor_reduce(st6[:, 2:3].unsqueeze(2),
                                    z0.unsqueeze(1), axis=AX.X, op=OP.add)
            nc.vector.tensor_copy(st6[:, 3:4], st6[:, 1:2])
            nc.scalar.activation(dummy[:], z1, AF.Square,
                                 accum_out=st6[:, 4:5])
            nc.vector.tensor_reduce(st6[:, 5:6].unsqueeze(2),
                                    z1.unsqueeze(1), axis=AX.X, op=OP.add)
            ps6 = ps_s.tile([1, 6], F32, tag="pss")
            nc.tensor.matmul(ps6[:], ones_col[:], st6[:],
                             start=True, stop=True)
            m6 = evac(ps6[:], 1, 6, "m6")
            nc.sync.dma_start(mom_scr[ci:ci + 1, :], m6[:])
            mom23 = wk.tile([2, 3], F32, tag="mom23")
            nc.sync.dma_start(mom23[:],
                              mom_scr[ci, :].rearrange("(a b) -> a b", a=2))
            embed_stats(mom23, Wes[c], 2, HE, 6 + 2 * ci)

        nc.sync.dma_start(ar1_i[:], stats[:])
        if emulate_collectives:
            nc.sync.dma_start(ar1_o[:], ar1_i[:])
        else:
            nc.gpsimd.collective_compute("AllReduce", OP.add,
                                         replica_groups=GRP,
                                         ins=[ar1_i], outs=[ar1_o])
        sts = ctile([128, 16], F32, "sts")
        nc.sync.dma_start(sts[:], ar1_o[:])

        def bn_vecs(src, scol, n, gc, bc, hh, tag):
            inv = 1.0 / n
            m = wk.tile([hh, 1], F32, tag=f"m{tag}")
            nc.vector.tensor_scalar_mul(m[:], src[0:hh, scol:scol + 1], inv)
            v = wk.tile([hh, 1], F32, tag=f"v{tag}")
            nc.vector.tensor_scalar_mul(v[:], src[0:hh, scol + 1:scol + 2],
                                        inv)
            msq = wk.tile([hh, 1], F32, tag=f"q{tag}")
            nc.vector.tensor_tensor(msq[:], m[:], m[:], op=OP.mult)
            nc.vector.tensor_tensor(v[:], v[:], msq[:], op=OP.subtract)
            nc.vector.tensor_scalar_add(v[:], v[:], EPS)
            sd = wk.tile([hh, 1], F32, tag=f"s{tag}")
            nc.scalar.activation(sd[:], v[:], AF.Sqrt)
            rsd = wk.tile([hh, 1], F32, tag=f"r{tag}")
            nc.vector.reciprocal(rsd[:], sd[:])
            sc = ctile([hh, 1], F32, f"sc{tag}")
            nc.vector.tensor_tensor(sc[:], rsd[:], gc, op=OP.mult)
            sh = ctile([hh, 1], F32, f"sh{tag}")
            nc.vector.tensor_tensor(sh[:], m[:], sc[:], op=OP.mult)
            nc.vector.tensor_tensor(sh[:], bc, sh[:], op=OP.subtract)
            return sc, sh

        sc0, sh0 = bn_vecs(sts, 0, B * D, bcol["b0_g"], bcol["b0_b"], H, "b0")
        sc1, sh1 = bn_vecs(sts, 2, B * N2, bcol["b1_g"], bcol["b1_b"], H, "b1")
        sc2, sh2 = bn_vecs(sts, 4, B * N2, bcol["b2_g"], bcol["b2_b"], H, "b2")
        sce, she = {}, {}
        sce["d"], she["d"] = bn_vecs(sts, 6, B * NA * NA, bcol["b3_g"],
                                     bcol["b3_b"], HE, "b3")
        sce["r"], she["r"] = bn_vecs(sts, 8, B * NA * NA, bcol["b4_g"],
                                     bcol["b4_b"], HE, "b4")

        # ---------- A10: lhsT_aug (65 rows) + prelu bias columns ----------
        laug, biasc = {}, {}
        for c in "dr":
            t = ctile([NROW, 9 * 128], F16, f"laug{c}")
            for k in "apd":
                for l in range(L):
                    w0 = wvcol(k, l)
                    nc.vector.tensor_scalar(t[0:64, w0:w0 + 128],
                                            wesl(k, l), sce[c][:], None,
                                            op0=OP.mult)
            nc.vector.memset(t[64:65, :], 200.0)
            laug[c] = t
            psb = ps_s.tile([128, 9], F32, tag="pss")
            for k in "apd":
                for l in range(L):
                    i = KIDX[k] * 3 + l
                    nc.tensor.matmul(psb[:, i:i + 1], wesl(k, l), she[c][:],
                                     start=True, stop=True)
            bt = ctile([128, 9], F32, f"biasc{c}")
            nc.vector.tensor_scalar_add(bt[:], psb[:], -200.0)
            biasc[c] = bt

        # ---------- A9: node embeddings -> xT0 ----------
        xT0 = xpool.tile([H, BL * NA], F16, tag="xT")
        dsv = IT[0:5, 0:204].rearrange("p (g n) -> p g n", g=BL)
        x0v = xT0[:].rearrange("p (g n) -> p g n", g=BL)
        ps = ps_s.tile([H, BL * D], F32, tag="pss")
        nc.tensor.matmul(ps[:], Wsl["W0"], dsv[:, :, 0:D], start=True,
                         stop=True)
        nc.vector.tensor_scalar(
            x0v[:, :, 0:D], ps[:].rearrange("p (g n) -> p g n", g=BL),
            sc0[:], sh0[:], op0=OP.mult, op1=OP.add)
        ps = ps_s.tile([H, BL * N2], F32, tag="pss")
        nc.tensor.matmul(ps[:], Wsl["W1"],
                         IT[0:10, 204:304].rearrange("p (g n) -> p g n", g=BL),
                         start=True, stop=True)
        nc.vector.tensor_scalar(
            x0v[:, :, D:D + N2], ps[:].rearrange("p (g n) -> p g n", g=BL),
            sc1[:], sh1[:], op0=OP.mult, op1=OP.add)
        ps = ps_s.tile([H, BL * N2], F32, tag="pss")
        nc.tensor.matmul(ps[:], Wsl["W2"], dsv[:, :, D + N2:NA],
                         start=True, stop=True)
        nc.vector.tensor_scalar(
            x0v[:, :, D + N2:NA], ps[:].rearrange("p (g n) -> p g n", g=BL),
            sc2[:], sh2[:], op0=OP.mult, op1=OP.add)

        for f_ in embed_iters("d", "AD"):
            f_()
        for f_ in embed_iters("r", "AD"):
            f_()

        def ghook_fn():
            pass

        # ---------- conv layers ----------
        ucount = [0]

        def mm_evac(w, rhs_ap, wid, tag):
            ps = ps_s.tile([H, wid], F32, tag="pss")
            nc.tensor.matmul(ps[:], w, rhs_ap, start=True, stop=True)
            t = xpool.tile([H, wid], dt.float32r, tag=tag)
            nc.scalar.copy(t[:], ps[:])
            th = xpool.tile([H, wid], F16, tag=tag + "h")
            nc.vector.tensor_copy(th[:], t[:])
            return t, th

        def prep_chain(c, l, xTin):
            xv = xTin[:].rearrange("p (g n) -> p g n", g=BL)
            xall, xallh = mm_evac(wv[("a", l)], xTin[:], BL * NA, "xa")
            pick, pickh = mm_evac(wv[("p", l)], xv[:, :, D:D + N2],
                                  BL * N2, "xp")
            deli, delih = mm_evac(wv[("d", l)], xv[:, :, D + N2:NA],
                                  BL * N2, "xd")
            return [("a", xall, xallh, NA, _chunks_full()),
                    ("p", pick, pickh, N2, _chunks_sub()),
                    ("d", deli, delih, N2, _chunks_sub())]

        def fetch_eg(c, k, g, S):
            Eg = epool.tile([NROW, S * S], F16, tag=f"Eg{k}", bufs=2)
            if k == "a":
                half = (S // 2) * S
                nc.sync.dma_start(Eg[:, 0:half],
                                  E_st[c][:, g, 0:S // 2, :])
                nc.sync.dma_start(Eg[:, half:S * S],
                                  E_st[c][:, g, S // 2:S, :])
            elif k == "p":
                nc.sync.dma_start(Eg[:, 0:S * S],
                                  E_st[c][:, g, D:D + N2, D:D + N2])
            else:
                nc.sync.dma_start(Eg[:, 0:S * S],
                                  E_st[c][:, g, D + N2:NA, D + N2:NA])
            return Eg

        def emit_units(c, l, cfg, hooks, eg0=None):
            i = 0
            for k_, xk_, xkh_, S_, chunks_ in cfg:
                for g_ in range(BL):
                    emit_unit(c, l, k_, xk_, xkh_, S_, chunks_, g_,
                              eg0 if i == 0 else None)
                    if i in hooks:
                        hooks[i]()
                    i += 1

        flushq = []
        ghook = [None]

        def drainq(keep=0):
            while len(flushq) > keep:
                flushq.pop(0)()

        def emit_unit(c, l, k, xk, xkh, S, chunks, g, eg=None):
            ilen = S
            uc = ucount[0]
            ucount[0] += 1
            la = laug[c][:, wvcol(k, l):wvcol(k, l) + 128]
            bias_ap = biasc[c][:, KIDX[k] * 3 + l:KIDX[k] * 3 + l + 1]
            wi, wjt = wi_s[(k, l)], wj_s[(k, l)]
            Eg = eg if eg is not None else fetch_eg(c, k, g, S)

            Dt = wk.tile([H, S], F16, tag="Dt")
            Nt = wk.tile([H, S], F16, tag="Nt")
            grps = _groups(chunks)
            xkb = xkh[:, g * S:g * S + ilen]
            hl = ilen // 2
            pend = []
            tgp = None

            def make_flush(Pgt, mypend):
                def fl():
                    for half, grp, fd, gix in mypend:
                        base_off = half * 1536
                        runs = []
                        for ki, (j0, nj) in enumerate(grp):
                            if runs and runs[-1][2] == nj:
                                runs[-1][1] += 1
                            else:
                                runs.append([ki, 1, nj, j0])
                        wg = fpool.tile([128, 1536], F16, tag="wg", bufs=2)
                        pf = fpool.tile([128, 768], F16, tag="pf", bufs=2)
                        wf = fpool.tile([128, 768], F16, tag="wf", bufs=2)
                        for k0, nk, nj, j0r in runs:
                            pbase = Pgt[:, base_off + k0 * 512:
                                        base_off + (k0 + nk) * 512]
                            wbase = wg[:, k0 * 512:(k0 + nk) * 512]
                            pv4 = pbase.rearrange("p (k r) -> p k r", k=nk)\
                                [:, :, 0:nj * ilen].rearrange(
                                "p k (j i) -> p k j i", i=ilen)
                            wv4 = wbase.rearrange("p (k r) -> p k r", k=nk)\
                                [:, :, 0:nj * ilen].rearrange(
                                "p k (j i) -> p k j i", i=ilen)
                            xb4 = xkb.unsqueeze(1).unsqueeze(1)\
                                .broadcast_to([H, nk, nj, ilen])
                            nc.vector.tensor_tensor(wv4, pv4, xb4, op=OP.mult)
                            pf4 = pf[:, k0 * 256:k0 * 256 + nk * nj * hl]\
                                .rearrange("p (k j i) -> p k j i", k=nk, j=nj)
                            wf4 = wf[:, k0 * 256:k0 * 256 + nk * nj * hl]\
                                .rearrange("p (k j i) -> p k j i", k=nk, j=nj)
                            nc.gpsimd.tensor_tensor(
                                pf4, pv4[:, :, :, 0:hl],
                                pv4[:, :, :, hl:2 * hl], op=OP.add)
                            wf_pool = (k == "a") and (gix % 2 == 0)
                            eng = nc.gpsimd if wf_pool else nc.vector
                            eng.tensor_tensor(
                                wf4, wv4[:, :, :, 0:hl],
                                wv4[:, :, :, hl:2 * hl], op=OP.add)
                            dv = Dt[:, j0r:j0r + nk * nj].rearrange(
                                "p (k j) -> p k j", k=nk)
                            nv = Nt[:, j0r:j0r + nk * nj].rearrange(
                                "p (k j) -> p k j", k=nk)
                            with nc.allow_low_precision("fp16 softmax sums"):
                                nc.vector.tensor_reduce(dv, pf4, axis=AX.X,
                                                        op=OP.add)
                                nc.vector.tensor_reduce(nv, wf4, axis=AX.X,
                                                        op=OP.add)
                return fl

            n_grps = len(grps)
            for gix, grp in enumerate(grps):
                half = gix % 2
                if half == 0:
                    tgp = fpool.tile([128, 3072], F16, tag="tg", bufs=2)
                j0g = grp[0][0]
                psg = ps_b.tile([128, 1536], F32, tag="psg")
                eoff = j0g * ilen
                for ki, (j0, nj) in enumerate(grp):
                    nc.tensor.matmul(
                        psg[:, ki * 512:ki * 512 + nj * ilen],
                        la, Eg[:, eoff:eoff + nj * ilen],
                        start=True, stop=False)
                    eoff += nj * ilen
                for ki, (j0, nj) in enumerate(grp):
                    a_rhs = xk[:, g * S + j0:g * S + j0 + nj]\
                        .unsqueeze(2).broadcast_to([H, nj, ilen])
                    nc.tensor.matmul(
                        psg[:, ki * 512:ki * 512 + nj * ilen],
                        wi[:], a_rhs, start=False, stop=False)
                b_base = xk[:, g * S:g * S + ilen]
                for ki, (j0, nj) in enumerate(grp):
                    b_rhs = b_base.unsqueeze(1).broadcast_to([H, nj, ilen])
                    nc.tensor.matmul(
                        psg[:, ki * 512:ki * 512 + nj * ilen],
                        wjt[:], b_rhs, start=False, stop=True)
                fd = (len(grp) - 1) * 512 + grp[-1][1] * ilen
                nc.scalar.activation(tgp[:, half * 1536:half * 1536 + fd],
                                     psg[:, 0:fd], AF.Prelu, alpha=SLOPE,
                                     bias=bias_ap)
                pend.append((half, grp, fd, gix))
                if ghook[0] is not None:
                    ghook[0]()
                if half == 1 or gix == n_grps - 1:
                    span = half * 1536 + fd
                    Pgp = fpool.tile([128, 3072], F16, tag="Pg", bufs=3)
                    nc.scalar.activation(Pgp[:, 0:span], tgp[:, 0:span],
                                         AF.Exp)
                    flushq.append(make_flush(Pgp, list(pend)))
                    pend.clear()
                    drainq(1)

            def tail():
                Di = wk.tile([H, S], F32, tag="Di")
                nc.vector.reciprocal(Di[:], Dt[:])
                og = wk.tile([H, S], F16, tag="og")
                nc.vector.tensor_tensor(og[:], Nt[:], Di[:], op=OP.mult)
                pst = ps_s.tile([S, H], F16, tag="pss")
                nc.tensor.transpose(pst[:], og[:], identh[:])
                nat2 = wk.tile([S, H], F16, tag="nat2")
                if uc % 2 == 0:
                    nc.scalar.copy(nat2[:], pst[:])
                else:
                    nc.vector.tensor_copy(nat2[:], pst[:])
                if k == "a":
                    off = pid * (BL * NA * H) + g * NA * H
                elif k == "p":
                    off = pid * (BL * N2 * H) + g * N2 * H + B * NA * H
                else:
                    off = pid * (BL * N2 * H) + g * N2 * H \
                        + B * (NA + N2) * H
                nc.sync.dma_start(
                    rs_in[c][bass.ds(off, S * H)].rearrange(
                        "(r h) -> r h", h=H), nat2[:])

            flushq.append(tail)
            drainq(1)

        def emit_rs(c, l):
            if emulate_collectives:
                nc.sync.dma_start(rs_out[(c, l)][:], rs_in[c][0:WIN * H])
            else:
                with nc.allow_low_precision("fp16 reduce-scatter"):
                    nc.gpsimd.collective_compute(
                        "ReduceScatter", OP.add, replica_groups=GRP,
                        ins=[rs_in[c]], outs=[rs_out[(c, l)]])

        def assemble(c, l):
            rsv = rs_out[(c, l)].rearrange("(r h) -> r h", h=H)
            xn = xpool.tile([H, BL * NA], F16, tag="xT")
            for g in range(BL):
                n1 = wk.tile([128, H], F16, tag="asm")
                nc.sync.dma_start(n1[:], rsv[g * 202:g * 202 + 128, :])
                n2 = wk.tile([128, H], F16, tag="asm")
                nc.sync.dma_start(
                    n2[0:74, :], rsv[g * 202 + 128:g * 202 + 202, :])
                p1 = ps_s.tile([H, 128], F16, tag="pss")
                nc.tensor.transpose(p1[:], n1[:], identh[:])
                p2 = ps_s.tile([H, 128], F16, tag="pss")
                nc.tensor.transpose(p2[:, 0:74], n2[0:74, :],
                                    identh[0:74, 0:74])
                xb = wk.tile([H, 202], F16, tag="xb")
                nc.scalar.copy(xb[:, 0:128], p1[:])
                nc.scalar.copy(xb[:, 128:202], p2[:, 0:74])
                nc.scalar.copy(xn[:, g * NA:g * NA + D], xb[:, 0:D])
                nc.vector.tensor_tensor(xn[:, g * NA + D:g * NA + NA],
                                        xb[:, D:NA], xb[:, NA:202],
                                        op=OP.add)
            return xn

        xT = {"d": xT0, "r": xT0}
        cfgs = {}
        egpre = {}
        cfgs[("d", 0)] = prep_chain("d", 0, xT0)
        egpre[("d", 0)] = fetch_eg("d", "a", 0, NA)

        def unit_closures(c, l):
            out = []
            i = 0
            for k_, xk_, xkh_, S_, chunks_ in cfgs[(c, l)]:
                for g_ in range(BL):
                    def fn(k=k_, xk=xk_, xkh=xkh_, S=S_, ch=chunks_, g=g_,
                           ii=i, cc=c, ll=l):
                        emit_unit(cc, ll, k, xk, xkh, S, ch, g,
                                  egpre.pop((cc, ll), None) if ii == 0
                                  else None)
                    out.append(fn)
                    i += 1
            return out

        st2 = {c: ctile([128, 4], F32, f"st2{c}") for c in "dr"}
        for c_ in "dr":
            nc.vector.memset(st2[c_][:], 0.0)
        zt = {}

        def ff_chain(c, xc):
            ps1 = ps_s.tile([H, BL * NA], F32, tag="pss")
            nc.tensor.matmul(ps1[:], ffw1, xc[:], start=True, stop=True)
            r = wk.tile([H, BL * NA], F16, tag="ffr")
            nc.scalar.activation(r[:], ps1[:], AF.Relu, bias=bcol["ff_b1"])
            ps2 = ps_s.tile([H, BL * NA], F32, tag="pss")
            nc.tensor.matmul(ps2[:], ffw2, r[:], start=True, stop=True)
            z = xpool.tile([H, BL * NA], F32, tag="zt")
            nc.vector.scalar_tensor_tensor(z[:], ps2[:], bcol["ff_b2"],
                                           xc[:], op0=OP.add, op1=OP.add)
            zt[c] = z
            nc.vector.tensor_reduce(st2[c][:, 0:1], z[:],
                                    axis=AX.X, op=OP.add)
            sq = fpool.tile([H, BL * NA], F16, tag="sq")
            nc.scalar.activation(sq[:], z[:], AF.Square,
                                 accum_out=st2[c][:, 1:2])
            nc.sync.dma_start(ar2_i[c][:], st2[c][:])
            if emulate_collectives:
                nc.sync.dma_start(ar2_o[c][:], ar2_i[c][:])
            else:
                nc.gpsimd.collective_compute("AllReduce", OP.add,
                                             replica_groups=GRP,
                                             ins=[ar2_i[c]], outs=[ar2_o[c]])
            st2o = ctile([128, 4], F32, f"st2o{c}")
            nc.sync.dma_start(st2o[:], ar2_o[c][:])
            sc, sh = bn_vecs(st2o, 0, B * NA, bcol["bn_g"],
                             bcol["bn_b"], H, f"ff{c}")
            oT = wk.tile([H, BL * NA], F32, tag="oT")
            nc.vector.tensor_scalar(oT[:], zt[c][:], sc[:], sh[:],
                                    op0=OP.mult, op1=OP.add)
            for g in range(BL):
                pso = ps_s.tile([NA, H], F32, tag="pss")
                nc.tensor.transpose(pso[:], oT[:, g * NA:(g + 1) * NA],
                                    ident[:])
                on = wk.tile([NA, H], F32, tag="on")
                nc.scalar.copy(on[:], pso[:])
                nc.sync.dma_start(o_out[c][g], on[:])

        phases = [(c, l) for l in range(L) for c in "dr"]
        carried = False
        pending_rs = [None]
        for pi, (c, l) in enumerate(phases):
            U = unit_closures(c, l)
            start = 1 if carried else 0
            carried = False
            nxt = phases[pi + 1] if pi + 1 < len(phases) else None
            for ui in range(start, 5):
                U[ui]()
                if ui == 1 and pending_rs[0] is not None:
                    drainq(0)
                    pending_rs[0]()
                    pending_rs[0] = None
                    if nxt is not None and nxt[1] > 0:
                        xT[nxt[0]] = assemble(nxt[0], nxt[1] - 1)
                    if nxt is None:
                        xT["d"] = assemble("d", L - 1)
                if pi == 0 and ui == 1:
                    xT["r"] = xT0
                if nxt is not None and ui == 2:
                    cfgs[nxt] = prep_chain(nxt[0], nxt[1], xT[nxt[0]])
                    egpre[nxt] = fetch_eg(nxt[0], "a", 0, NA)
                if nxt is None and ui == 4:
                    ff_chain("d", xT["d"])
            if nxt is not None:
                unit_closures(*nxt)[0]()
                carried = True
            U[5]()
            pending_rs[0] = (lambda cc=c, ll=l: emit_rs(cc, ll))
        drainq(0)
        pending_rs[0]()

        # ---------- FF head ----------
        xTr = assemble("r", L - 1)
        ff_chain("r", xTr)

    nc.compile()
    return nc


def _prep_core(inputs, core):
    sl = slice(2 * core, 2 * core + 2)
    x = np.asarray(inputs["x"])[sl]
    dem = np.asarray(inputs["demand"])[sl]
    tw = np.asarray(inputs["time_window"])[sl]
    ds = np.concatenate([x, dem, tw], -1).astype(np.float32)
    dsT = np.ascontiguousarray(ds.transpose(2, 0, 1).reshape(5, BL * NA))
    pkin = np.concatenate([ds[:, D:D + N2], ds[:, D + N2:NA]], -1)
    pkinT = np.ascontiguousarray(pkin.transpose(2, 0, 1).reshape(10, BL * N2))

    IT = np.zeros((128, _IW), np.float32)
    IT[0:5, 0:204] = dsT
    IT[0:10, 204:304] = pkinT
    IT[0:4, 304:309] = ds[:, :D].reshape(BL * D, 5)
    IT[0:4, 309] = 1.0
    IT[0:100, 312:322] = pkin.reshape(BL * N2, 10)
    IT[0:100, 322] = 1.0
    IT[0:100, 324:329] = ds[:, D + N2:NA].reshape(BL * N2, 5)
    IT[0:100, 329] = 1.0

    eT4 = np.zeros((4, COLS), np.float16)
    ms = {}
    for ci, (c2, key_e, key_m) in enumerate(
            (("d", "edge_attr_d", "mask_adjacency_d"),
             ("r", "edge_attr_r", "mask_adjacency_r"))):
        ea = np.asarray(inputs[key_e])[sl].reshape(BL, NA, NA, 2)
        eT4[2 * ci:2 * ci + 2] = ea.transpose(3, 0, 2, 1).reshape(2, COLS)
        tmp = np.zeros((163 * 128, 3), np.float32)
        tmp[:BL * NA * NA, :2] = ea.reshape(BL * NA * NA, 2)
        tmp[:BL * NA * NA, 2] = 1.0
        nat = tmp.reshape(163, 128, 3).transpose(1, 0, 2).reshape(128, 489)
        c0 = 332 if c2 == "d" else 824
        IT[:, c0:c0 + 489] = nat
        mm = np.asarray(inputs[key_m])[sl].reshape(BL, NA, NA)
        ms[c2] = np.ascontiguousarray(
            mm.transpose(0, 2, 1).reshape(BL * NA * NA)).astype(np.float16)

    CT = np.zeros((128, _CW), np.float32)
    CTH = np.zeros((128, _CWH), np.float16)
    KI = {"a": 0, "p": 1, "d": 2}
    Wvl = {"a": np.asarray(inputs["Wvla"], np.float32),
           "p": np.asarray(inputs["Wvlp"], np.float32),
           "d": np.asarray(inputs["Wvld"], np.float32)}
    Wgx = {"a": np.asarray(inputs["Wga"], np.float32),
           "p": np.asarray(inputs["Wgp"], np.float32),
           "d": np.asarray(inputs["Wgd"], np.float32)}
    for k in "apd":
        for l in range(L):
            w0 = (KI[k] * 3 + l) * 128
            CTH[:, w0:w0 + 128] = Wvl[k][l]
            CTH[:, 1152 + w0:1152 + w0 + 128] = Wgx[k][l, 0:H, :]
            CTH[:, 2304 + w0:2304 + w0 + 128] = Wgx[k][l, H:2 * H, :]
            CT[0:64, w0:w0 + 128] = Wgx[k][l, 2 * H:2 * H + HE, :]
    CTH[:, 3456:3584] = np.asarray(inputs["ff_w1"], np.float32)
    CTH[:, 3584:3712] = np.asarray(inputs["ff_w2"], np.float32)
    CT[0:5, 1152:1280] = np.asarray(inputs["W0"], np.float32)
    CT[0:10, 1280:1408] = np.asarray(inputs["W1"], np.float32)
    CT[0:5, 1408:1536] = np.asarray(inputs["W2"], np.float32)
    CT[0:2, 1536:1600] = np.asarray(inputs["W3"], np.float32)
    CT[0:2, 1600:1664] = np.asarray(inputs["W4"], np.float32)
    for i, nmv in enumerate(["b0_g", "b0_b", "b1_g", "b1_b", "b2_g", "b2_b",
                             "b3_g", "b3_b", "b4_g", "b4_b",
                             "ff_b1", "ff_b2", "bn_g", "bn_b"]):
        v = np.asarray(inputs[nmv], np.float32)
        CT[0:v.shape[0], 1664 + i] = v

    return dict(consts=CT, consth=CTH, inputs=IT, eT4=eT4,
                m_d=ms["d"], m_r=ms["r"])


def get_in_maps(inputs):
    return [_prep_core(inputs, c) for c in range(NCORE)]


def kernel(**inputs):
    if "nc" not in _CACHE:
        _CACHE["nc"] = build()
    nc = _CACHE["nc"]
    from concourse.bass_utils import run_bass_kernel_spmd
    in_maps = get_in_maps(inputs)
    res = run_bass_kernel_spmd(nc, in_maps, list(range(NCORE))).results
    od = np.concatenate([res[c]["o_d"] for c in range(NCORE)], 0)
    orr = np.concatenate([res[c]["o_r"] for c in range(NCORE)], 0)
    return od, orr
